# revision 1
# baseline (speedup 1.0000x reference)
"""DarkCapsuleNet on 8 Trainium2 NeuronCores.

Data-parallel over batch (B=8, one image per core). The conv+BN+LReLU
backbone runs per core on its image; BN batch statistics are combined
across cores with tiny AllReduces (per-channel [mean, E[x^2]] sums). The
capsule-routing stage is independent per (grid-cell, image), so each core
routes its own 16 cells entirely in SBUF.

Convs are direct convolutions: matmuls accumulated over kernel offsets with
input channels on the contraction dim, bf16 operands, fp32 PSUM. Priors use
a block-diagonal lhsT built on-chip with one masked DVE multiply per tile,
so the 8-wide capsule contraction still runs as full 128-wide matmuls.
"""

import numpy as np
import ml_dtypes


class _PhaseStop(Exception):
    def __init__(self, nc):
        self.nc = nc

N_CLASSES = 43
KO = N_CLASSES * 21  # 903
EPS = 1e-5
NCORES = 8

_BF16 = ml_dtypes.bfloat16


# ---------------------------------------------------------------------------
# Workaround: this walrus build accepts at most ONE sem wait on a TPB_CTRL
# Drain instruction; Tile's epilogue drain carries one wait per HW-DMA queue.
# Split the extra waits onto standalone SP nops (same engine, before the
# all-engine barrier, so semantics are unchanged).
# ---------------------------------------------------------------------------
def _install_tile_drain_fix():
    import concourse.tile as tile_mod
    import concourse.mybir as mybir
    from concourse.vector_clock import ScopedClock

    if getattr(tile_mod.TileContext, "_drain_fix_installed", False):
        return

    def _patched(self, tick_clock, wait_clock):
        drain_inst = self.nc.sync.drain()
        wait_clock.add_sem_waits(
            drain_inst.ins, ScopedClock({None: tick_clock.global_clock})
        )
        raw = drain_inst.ins
        si = getattr(raw, "sync_info", None)
        if si is not None and si.on_wait is not None and len(si.on_wait) > 1:
            waits = list(si.on_wait)
            si.on_wait = waits[-1:]
            for w in waits[:-1]:
                nop = self.nc.sync.nop(nofuse=True, hint="split_drain_wait")
                nsi = getattr(nop.ins, "sync_info", None)
                if nsi is None:
                    nop.ins.sync_info = mybir.SyncInfo(on_update=[], on_wait=[w])
                else:
                    nw = list(nsi.on_wait) if nsi.on_wait else []
                    nw.append(w)
                    nsi.on_wait = nw
        self.nc.all_engine_barrier()
        assert self.sems is not None
        popped = self.nc._tile_sem_poison_stack.pop()
        assert popped is self._sem_poison
        self.nc.clear_and_free_semaphores(list(self.sems.allocated().values()))
        self.nc.all_engine_barrier()

    tile_mod.TileContext._drain_and_barrier = _patched
    tile_mod.TileContext._drain_fix_installed = True


# ---------------------------------------------------------------------------
# Host-side layout prep
# ---------------------------------------------------------------------------
def _bf(x):
    return np.ascontiguousarray(np.asarray(x, np.float32).astype(_BF16))


def _im2col(img):
    # img (3,128,128) f32 -> (27,16384), rows (ci,ky,kx)
    xp = np.zeros((3, 130, 130), np.float32)
    xp[:, 1:129, 1:129] = img
    cols = np.empty((3, 3, 3, 128, 128), np.float32)
    for ky in range(3):
        for kx in range(3):
            cols[:, ky, kx] = xp[:, ky : ky + 128, kx : kx + 128]
    return cols.reshape(27, 16384)


def _prep_shared(d):
    c1h = np.asarray(d["c1w"], np.float32).reshape(128, 27).T.copy()
    c2h = np.asarray(d["c2w"], np.float32).transpose(2, 3, 1, 0).reshape(9, 128, 256)
    c2h = np.concatenate(list(c2h), axis=1)  # (128, 9*256)
    c3t = np.asarray(d["c3w"], np.float32).transpose(1, 2, 3, 0)  # (256,4,4,64)
    c3h = np.concatenate(
        [c3t[m * 128 : (m + 1) * 128].reshape(128, 16 * 64) for m in range(2)], axis=1
    )  # (128, 2048)
    c4h = np.asarray(d["c4w"], np.float32).transpose(1, 2, 3, 0).reshape(64, 16 * 128)
    c5h = np.asarray(d["c5w"], np.float32).transpose(1, 2, 3, 0).reshape(128, 16 * 256)

    rw = np.asarray(d["rw"], np.float32)  # (512,43,8,21)
    rt = rw.transpose(0, 2, 1, 3).reshape(512 * 8, KO)  # row = n*8+i
    # RT[t*128 + ns*8 + i] = rw[16t+ns, :, i, :]  -> same as rt row (16t+ns)*8+i
    # rt rows are already (n,i) with n major: n*8+i = (16t+ns)*8+i = t*128+ns*8+i ✓

    gb = np.zeros((128, 14), np.float32)
    gb[:, 0] = d["g1"]; gb[:, 1] = d["b1"]
    gb[:, 2] = d["g2"][:128]; gb[:, 3] = d["b2"][:128]
    gb[:, 4] = d["g2"][128:]; gb[:, 5] = d["b2"][128:]
    gb[:64, 6] = d["g3"]; gb[:64, 7] = d["b3"]
    gb[:, 8] = d["g4"]; gb[:, 9] = d["b4"]
    gb[:, 10] = d["g5"][:128]; gb[:, 11] = d["b5"][:128]
    gb[:, 12] = d["g5"][128:]; gb[:, 13] = d["b5"][128:]

    mask = np.zeros((128, 128), np.float32)
    for p in range(128):
        mask[p, (p >> 3) * 8 : (p >> 3) * 8 + 8] = 1.0
    selb = np.zeros((128, 8), np.float32)
    for p in range(128):
        selb[p, p & 7] = 1.0
    selr = np.zeros((8, 128), np.float32)  # [b, ns*8 + b]
    for ns in range(16):
        for b in range(8):
            selr[b, ns * 8 + b] = 1.0
    return dict(
        c1wT=_bf(c1h), c2wT=_bf(c2h), c3wT=_bf(c3h), c4wT=_bf(c4h), c5wT=_bf(c5h),
        RT=_bf(rt), gb=gb, MASK=_bf(mask), SELB=_bf(selb), SELB43=_bf(selb / 43.0),
        SELR=_bf(selr),
    )


# ---------------------------------------------------------------------------
# Bass program (identical on every core)
# ---------------------------------------------------------------------------
def _spill_extra_waits(nc):
    """This walrus codegen accepts at most one semaphore wait per TPB
    instruction. Tile can attach several. Move the extras onto fresh NoOp
    instructions inserted just before the owner on the same engine."""
    import concourse.mybir as mybir

    uid = [0]
    for f in nc.m.functions:
        for bb in f.blocks:
            il = bb.instructions
            out = []
            changed = False
            for inst in il:
                si = getattr(inst, "sync_info", None)
                waits = list(si.on_wait) if si is not None and si.on_wait else []
                if len(waits) > 1:
                    for w in waits[:-1]:
                        uid[0] += 1
                        nop = mybir.InstNoOp(name=f"waitspill-{uid[0]}", ins=[], outs=[])
                        nop.engine = inst.engine
                        nop.sync_info = mybir.SyncInfo(on_update=[], on_wait=[w])
                        out.append(nop)
                    si.on_wait = waits[-1:]
                    changed = True
                out.append(inst)
            if changed:
                bb.instructions = out


def _build_bass(phase_limit=99):
    import concourse.bass as bass
    import concourse.mybir as mybir
    from concourse import tile

    _install_tile_drain_fix()

    F32 = mybir.dt.float32
    BF16 = mybir.dt.bfloat16
    F16 = mybir.dt.float16
    ADD = mybir.AluOpType.add
    MULT = mybir.AluOpType.mult
    SUB = mybir.AluOpType.subtract
    ACTF = mybir.ActivationFunctionType
    AXX = mybir.AxisListType.X

    nc = bass.Bass(num_devices=NCORES)
    dp = nc.declare_dram_parameter
    i_xcol = dp("xcol", [27, 16384], BF16, isOutput=False)
    i_c1 = dp("c1wT", [27, 128], BF16, isOutput=False)
    i_c2 = dp("c2wT", [128, 2304], BF16, isOutput=False)
    i_c3 = dp("c3wT", [128, 2048], BF16, isOutput=False)
    i_c4 = dp("c4wT", [64, 2048], BF16, isOutput=False)
    i_c5 = dp("c5wT", [128, 4096], BF16, isOutput=False)
    i_rt = dp("RT", [4096, KO], BF16, isOutput=False)
    i_gb = dp("gb", [128, 14], F32, isOutput=False)
    i_mask = dp("MASK", [128, 128], BF16, isOutput=False)
    i_selb = dp("SELB", [128, 8], BF16, isOutput=False)
    i_selb43 = dp("SELB43", [128, 8], BF16, isOutput=False)
    i_selr = dp("SELR", [8, 128], BF16, isOutput=False)
    o_out = dp("out", [16, KO], F32, isOutput=True)


    with tile.TileContext(nc) as tc:
        with tc.tile_pool(name="const", bufs=1) as const, \
             tc.tile_pool(name="dram", bufs=1, space="DRAM") as dram:
            t_gb = const.tile([128, 14], F32)
            t_mask = const.tile([128, 128], BF16)
            t_selb = const.tile([128, 8], BF16)
            t_selb43 = const.tile([128, 8], BF16)
            t_selr = const.tile([8, 128], BF16)
            h5 = [const.tile([128, 256], BF16, tag=f"h5_{m}", name=f"h5_{m}") for m in range(2)]
            t_st6 = const.tile([128, 32 * 6], F32)
            t_mv = const.tile([128, 4], F32)
            t_ab = const.tile([128, 4], F32)
            t_sc = const.tile([128, 2], F32)
            for t, i in [(t_gb, i_gb), (t_mask, i_mask), (t_selb, i_selb),
                         (t_selb43, i_selb43), (t_selr, i_selr)]:
                nc.sync.dma_start(t[:], i[:])

            ar_in = [dram.tile([128, 4], F32, tag=f"ari{i}", name=f"ari{i}") for i in range(5)]
            ar_out = [dram.tile([128, 4], F32, tag=f"aro{i}", name=f"aro{i}") for i in range(5)]

            def bn_allreduce(layer, nch_tiles, npart):
                """t_mv holds per-core [m0,v0,m1,v1]; leaves [a0,b0,a1,b1] in t_ab."""
                for mt in range(nch_tiles):
                    m = t_mv[:npart, 2 * mt : 2 * mt + 1]
                    v = t_mv[:npart, 2 * mt + 1 : 2 * mt + 2]
                    s1 = t_sc[:npart, 0:1]
                    nc.scalar.activation(s1, m, ACTF.Square)
                    nc.vector.tensor_tensor(v, v, s1, ADD)  # v := E[x^2] local
                nc.sync.dma_start(ar_in[layer][:], t_mv[:])
                nc.gpsimd.collective_compute(
                    "AllReduce", ADD,
                    ins=[ar_in[layer][:]], outs=[ar_out[layer][:]],
                    replica_groups=[list(range(NCORES))],
                )
                nc.sync.dma_start(t_mv[:], ar_out[layer][:])
                for mt in range(nch_tiles):
                    m = t_mv[:npart, 2 * mt : 2 * mt + 1]
                    q = t_mv[:npart, 2 * mt + 1 : 2 * mt + 2]
                    a = t_ab[:npart, 2 * mt : 2 * mt + 1]
                    b = t_ab[:npart, 2 * mt + 1 : 2 * mt + 2]
                    s1 = t_sc[:npart, 0:1]
                    nc.vector.tensor_scalar_mul(m, m, 1.0 / NCORES)
                    nc.vector.tensor_scalar_mul(q, q, 1.0 / NCORES)
                    nc.scalar.activation(s1, m, ACTF.Square)
                    nc.vector.tensor_tensor(q, q, s1, SUB)       # gvar
                    nc.vector.tensor_scalar_add(q, q, EPS)
                    nc.vector.reciprocal(s1, q)
                    nc.scalar.activation(s1, s1, ACTF.Sqrt)      # rsqrt(var+eps)
                    gcol = (0, 2, 6, 8, 10)[layer] + 2 * mt
                    nc.vector.tensor_tensor(a, t_gb[:npart, gcol : gcol + 1], s1, MULT)
                    nc.vector.tensor_tensor(s1, a, m, MULT)
                    nc.vector.tensor_tensor(b, t_gb[:npart, gcol + 1 : gcol + 2], s1, SUB)

            def lrelu_apply(view, scale, bias):
                nc.scalar.activation(view, view, ACTF.Prelu,
                                     bias=bias, scale=scale, alpha=0.1)

            # ================= conv backbone =================
            with tc.tile_pool(name="wpool", bufs=1) as wp, \
                 tc.tile_pool(name="xpool", bufs=1) as xp, \
                 tc.tile_pool(name="acts", bufs=1) as acts, \
                 tc.tile_pool(name="cpsum", bufs=4, space="PSUM") as cpsum:
                t_c2 = wp.tile([128, 2304], BF16)
                t_c3 = wp.tile([128, 2048], BF16)
                t_c4 = wp.tile([64, 2048], BF16)
                t_c5 = wp.tile([128, 4096], BF16)
                t_c1 = xp.tile([27, 128], BF16)
                t_xcol = xp.tile([27, 16384], BF16)
                nc.sync.dma_start(t_c1[:], i_c1[:])
                for ch in range(4):
                    nc.sync.dma_start(t_xcol[:, ch * 4096 : (ch + 1) * 4096],
                                      i_xcol[:, ch * 4096 : (ch + 1) * 4096])

                h1 = acts.tile([128, 130 * 130], BF16)
                h2 = [acts.tile([128, 130 * 130], BF16, tag=f"h2_{m}", name=f"h2_{m}") for m in range(2)]
                h3 = acts.tile([64, 66 * 66], BF16)
                h4 = acts.tile([128, 34 * 34], BF16)

                def zero_border(tile_ap, H):
                    v = tile_ap.rearrange("p (a b) -> p a b", b=H)
                    nc.gpsimd.memset(v[:, 0:1, :], 0.0)
                    nc.gpsimd.memset(v[:, H - 1 : H, :], 0.0)
                    nc.gpsimd.memset(v[:, 1 : H - 1, 0:1], 0.0)
                    nc.gpsimd.memset(v[:, 1 : H - 1, H - 1 : H], 0.0)

                zero_border(h1[:], 130)
                zero_border(h2[0][:], 130)
                zero_border(h2[1][:], 130)
                zero_border(h3[:], 66)
                zero_border(h4[:], 34)

                # ---- conv1 ----
                for nt in range(32):
                    ps = cpsum.tile([128, 512], F32, tag="cps")
                    nc.tensor.matmul(ps[:], t_c1[:],
                                     t_xcol[:, nt * 512 : (nt + 1) * 512],
                                     start=True, stop=True)
                    intr = h1[:].rearrange("p (a b) -> p a b", b=130)[
                        :, 1 + nt * 4 : 5 + nt * 4, 1:129]
                    nc.scalar.activation(
                        intr, ps[:].rearrange("p (a b) -> p a b", b=128), ACTF.Copy)
                    nc.vector.bn_stats(t_st6[:, nt * 6 : nt * 6 + 6], ps[:])
                for t, i in [(t_c2, i_c2), (t_c3, i_c3), (t_c4, i_c4),
                             (t_c5, i_c5)]:
                    nc.sync.dma_start(t[:], i[:])
                nc.vector.bn_aggr(t_mv[:, 0:2],
                                  t_st6[:].rearrange("p (g s) -> p g s", s=6))
                bn_allreduce(0, 1, 128)
                h1v = h1[:].rearrange("p (a b) -> p a b", b=130)
                for c4_ in range(4):
                    lrelu_apply(h1v[:, 1 + 32 * c4_ : 33 + 32 * c4_, 1:129],
                                t_ab[:, 0:1], t_ab[:, 1:2])

                # ---- conv2 ----
                if phase_limit < 2:
                    raise _PhaseStop(nc)
                for m in range(2):
                    for nt in range(32):
                        ps = cpsum.tile([128, 512], F32, tag="cps")
                        for off in range(9):
                            ky, kx = off // 3, off % 3
                            rhs = h1v[:, ky + nt * 4 : ky + nt * 4 + 4, kx : kx + 128]
                            nc.tensor.matmul(
                                ps[:],
                                t_c2[:, off * 256 + m * 128 : off * 256 + m * 128 + 128],
                                rhs, start=(off == 0), stop=(off == 8))
                        intr = h2[m][:].rearrange("p (a b) -> p a b", b=130)[
                            :, 1 + nt * 4 : 5 + nt * 4, 1:129]
                        nc.scalar.activation(
                            intr, ps[:].rearrange("p (a b) -> p a b", b=128), ACTF.Copy)
                        nc.vector.bn_stats(t_st6[:, nt * 6 : nt * 6 + 6], ps[:])
                    nc.vector.bn_aggr(t_mv[:, 2 * m : 2 * m + 2],
                                      t_st6[:].rearrange("p (g s) -> p g s", s=6))
                bn_allreduce(1, 2, 128)
                h2v = [h2[m][:].rearrange("p (a b) -> p a b", b=130) for m in range(2)]
                for m in range(2):
                    for c4_ in range(4):
                        lrelu_apply(h2v[m][:, 1 + 32 * c4_ : 33 + 32 * c4_, 1:129],
                                    t_ab[:, 2 * m : 2 * m + 1],
                                    t_ab[:, 2 * m + 1 : 2 * m + 2])

                # ---- conv3 ----
                if phase_limit < 3:
                    raise _PhaseStop(nc)
                for nt in range(8):
                    ps = cpsum.tile([128, 512], F32, tag="cps")
                    first = True
                    for m in range(2):
                        for off in range(16):
                            ky, kx = off // 4, off % 4
                            rhs = h2v[m][:, ky + nt * 16 : ky + nt * 16 + 15 : 2,
                                         kx : kx + 127 : 2]
                            nc.tensor.matmul(
                                ps[:64, :],
                                t_c3[:, (m * 16 + off) * 64 : (m * 16 + off) * 64 + 64],
                                rhs, start=first, stop=(m == 1 and off == 15))
                            first = False
                    intr = h3[:].rearrange("p (a b) -> p a b", b=66)[
                        :, 1 + nt * 8 : 9 + nt * 8, 1:65]
                    nc.scalar.activation(
                        intr, ps[:64, :].rearrange("p (a b) -> p a b", b=64), ACTF.Copy)
                    nc.vector.bn_stats(t_st6[:64, nt * 6 : nt * 6 + 6], ps[:64, :])
                nc.vector.bn_aggr(
                    t_mv[:64, 0:2],
                    t_st6[:64, : 8 * 6].rearrange("p (g s) -> p g s", s=6))
                bn_allreduce(2, 1, 64)
                h3v = h3[:].rearrange("p (a b) -> p a b", b=66)
                lrelu_apply(h3v[:, 1:65, 1:65], t_ab[:64, 0:1], t_ab[:64, 1:2])

                # ---- conv4 ----
                if phase_limit < 4:
                    raise _PhaseStop(nc)
                for nt in range(2):
                    ps = cpsum.tile([128, 512], F32, tag="cps")
                    for off in range(16):
                        ky, kx = off // 4, off % 4
                        rhs = h3v[:, ky + nt * 32 : ky + nt * 32 + 31 : 2, kx : kx + 63 : 2]
                        nc.tensor.matmul(ps[:], t_c4[:, off * 128 : off * 128 + 128],
                                         rhs, start=(off == 0), stop=(off == 15))
                    intr = h4[:].rearrange("p (a b) -> p a b", b=34)[
                        :, 1 + nt * 16 : 17 + nt * 16, 1:33]
                    nc.scalar.activation(
                        intr, ps[:].rearrange("p (a b) -> p a b", b=32), ACTF.Copy)
                    nc.vector.bn_stats(t_st6[:, nt * 6 : nt * 6 + 6], ps[:])
                nc.vector.bn_aggr(
                    t_mv[:, 0:2], t_st6[:, :12].rearrange("p (g s) -> p g s", s=6))
                bn_allreduce(3, 1, 128)
                h4v = h4[:].rearrange("p (a b) -> p a b", b=34)
                lrelu_apply(h4v[:, 1:33, 1:33], t_ab[:, 0:1], t_ab[:, 1:2])

                # ---- conv5 ----
                if phase_limit < 5:
                    raise _PhaseStop(nc)
                for m in range(2):
                    ps = cpsum.tile([128, 512], F32, tag="cps")
                    first = True
                    for off in range(16):
                        ky, kx = off // 4, off % 4
                        rhs = h4v[:, ky : ky + 31 : 2, kx : kx + 31 : 2]
                        nc.tensor.matmul(
                            ps[:, 0:256],
                            t_c5[:, off * 256 + m * 128 : off * 256 + m * 128 + 128],
                            rhs, start=first, stop=(off == 15))
                        first = False
                    nc.scalar.activation(h5[m][:], ps[:, 0:256], ACTF.Copy)
                    nc.vector.bn_stats(t_st6[:, m * 6 : m * 6 + 6], ps[:, 0:256])
                for m in range(2):
                    nc.vector.bn_aggr(
                        t_mv[:, 2 * m : 2 * m + 2],
                        t_st6[:, m * 6 : m * 6 + 6].rearrange("p (g s) -> p g s", s=6))
                bn_allreduce(4, 2, 128)
                for m in range(2):
                    lrelu_apply(h5[m][:], t_ab[:, 2 * m : 2 * m + 1],
                                t_ab[:, 2 * m + 1 : 2 * m + 2])

            if phase_limit < 6:
                raise _PhaseStop(nc)
            # ================= priors =================
            with tc.tile_pool(name="pri", bufs=1) as pri, \
                 tc.tile_pool(name="route", bufs=1) as rp, \
                 tc.tile_pool(name="scr", bufs=4) as scr:
                P = [[pri.tile([128, 8 * KO], BF16, tag=f"P{g}_{j}", name=f"P{g}_{j}")
                      for j in range(4)] for g in range(2)]

                def P_t(g, t):
                    j, tj = t // 8, t % 8
                    return P[g][j][:, tj * KO : tj * KO + KO]
                with tc.tile_pool(name="ppsum", bufs=3, space="PSUM") as ppsum:
                    for t in range(32):
                        h = t >> 3
                        w = (t >> 1) & 3
                        mblk = t & 1
                        rt_t = scr.tile([128, KO], BF16, tag="rt", bufs=8)
                        nc.sync.dma_start(rt_t[:], i_rt[t * 128 : (t + 1) * 128, :])
                        hb = h5[mblk][:].rearrange(
                            "p (hh gy gx ww) -> p hh gy gx ww",
                            hh=4, gy=4, gx=4)
                        for g in range(2):
                            g8 = scr.tile([128, 8], BF16, tag="g8")
                            src = hb[:, h : h + 1, 2 * g : 2 * g + 2, :, w : w + 1]
                            # (p,1,2,4,1) -> (p,2,4)
                            src = src.rearrange("p a b d e -> p (a b) (d e)")
                            nc.gpsimd.tensor_copy(
                                g8[:].rearrange("p (b d) -> p b d", b=2), src)
                            lt = scr.tile([128, 128], BF16, tag="lt")
                            nc.vector.tensor_tensor(
                                lt[:].rearrange("p (n b) -> p n b", b=8),
                                g8[:].rearrange("p (o e) -> p o e", o=1)
                                    .broadcast_to([128, 16, 8]),
                                t_mask[:].rearrange("p (n b) -> p n b", b=8),
                                MULT)
                            pp = ppsum.tile([128, KO], F32, tag="pps")
                            nc.tensor.matmul(pp[:, 0:512], lt[:], rt_t[:, 0:512],
                                             start=True, stop=True)
                            nc.tensor.matmul(pp[:, 512:KO], lt[:], rt_t[:, 512:KO],
                                             start=True, stop=True)
                            if (t & 3) == 0:
                                nc.vector.tensor_copy(P_t(g, t), pp[:])
                            else:
                                nc.scalar.activation(P_t(g, t), pp[:], ACTF.Copy)

                # ================= routing =================
                if phase_limit < 7:
                    raise _PhaseStop(nc)
                NG = 4   # tile-groups per cell-group (8 tiles each)
                GT = 8
                L = [[rp.tile([128, GT * 43], F16, tag=f"L{g}_{j}", name=f"L{g}_{j}")
                      for j in range(NG)] for g in range(2)]
                PR = [[rp.tile([128, GT * 43], BF16, tag=f"PR{g}_{j}", name=f"PR{g}_{j}")
                       for j in range(NG)] for g in range(2)]
                s_g = [rp.tile([8, KO], F32, tag=f"s_g{g}", name=f"s_g{g}") for g in range(2)]
                sq = [rp.tile([8, KO], F32, tag=f"sq{g}", name=f"sq{g}") for g in range(2)]
                sn = [rp.tile([8, 43], F32, tag=f"sn{g}", name=f"sn{g}") for g in range(2)]
                den = [rp.tile([8, 43], F32, tag=f"den{g}", name=f"den{g}") for g in range(2)]
                phi = [rp.tile([8, 43], F32, tag=f"phi{g}", name=f"phi{g}") for g in range(2)]
                out_f = [rp.tile([8, KO], F32, tag=f"of{g}", name=f"of{g}") for g in range(2)]
                out_bf = [rp.tile([8, KO], BF16, tag=f"ob{g}", name=f"ob{g}") for g in range(2)]
                out_rep = [rp.tile([128, KO], BF16, tag=f"orep{g}", name=f"orep{g}") for g in range(2)]
                for g in range(2):
                    for j in range(NG):
                        nc.vector.memset(L[g][j][:], 0.0)

                with tc.tile_pool(name="rpsum", bufs=2, space="PSUM") as rpsum:
                    for it in range(3):
                        for g in range(2):
                            if it > 0:
                                for j in range(NG):
                                    e8 = scr.tile([128, GT * 43], F16, tag="e8")
                                    nc.scalar.activation(e8[:], L[g][j][:], ACTF.Exp)
                                    r8 = scr.tile([128, GT], F32, tag="r8")
                                    nc.vector.tensor_reduce(
                                        r8[:], e8[:].rearrange("p (t k) -> p t k", k=43),
                                        AXX, ADD)
                                    nc.vector.reciprocal(r8[:], r8[:])
                                    nc.vector.tensor_tensor(
                                        PR[g][j][:].rearrange("p (t k) -> p t k", k=43),
                                        e8[:].rearrange("p (t k) -> p t k", k=43),
                                        r8[:].rearrange("p (t k) -> p t k", k=1)
                                            .broadcast_to([128, GT, 43]),
                                        MULT)
                            sp = rpsum.tile([8, KO], F32, tag="sps")
                            for t in range(32):
                                j, tj = t // GT, t % GT
                                if it == 0:
                                    rhs_t = P_t(g, t)
                                    lhs = t_selb43
                                else:
                                    tm = scr.tile([128, KO], BF16, tag="tm", bufs=6)
                                    teng = nc.gpsimd if (t & 3) == 3 else nc.vector
                                    teng.tensor_tensor(
                                        tm[:].rearrange("p (k o) -> p k o", o=21),
                                        P_t(g, t).rearrange("p (k o) -> p k o", o=21),
                                        PR[g][j][:, tj * 43 : tj * 43 + 43]
                                        .rearrange("p (k o) -> p k o", o=1)
                                        .broadcast_to([128, 43, 21]),
                                        MULT)
                                    rhs_t = tm[:]
                                    lhs = t_selb
                                nc.tensor.matmul(sp[:, 0:512], lhs[:], rhs_t[:, 0:512],
                                                 start=(t == 0), stop=(t == 31))
                                nc.tensor.matmul(sp[:, 512:KO], lhs[:], rhs_t[:, 512:KO],
                                                 start=(t == 0), stop=(t == 31))
                            nc.scalar.activation(s_g[g][:], sp[:], ACTF.Copy)
                        # squash: out = s * sqrt(sn)/(1+sn)
                        for g in range(2):
                            nc.scalar.activation(sq[g][:], s_g[g][:], ACTF.Square)
                            nc.vector.tensor_reduce(
                                sn[g][:], sq[g][:].rearrange("p (k o) -> p k o", o=21),
                                AXX, ADD)
                            nc.vector.tensor_scalar_add(den[g][:], sn[g][:], 1.0)
                            nc.vector.reciprocal(den[g][:], den[g][:])
                            nc.scalar.activation(phi[g][:], sn[g][:], ACTF.Sqrt)
                            nc.vector.tensor_tensor(phi[g][:], phi[g][:], den[g][:], MULT)
                            tgt = out_f[g] if it == 2 else out_bf[g]
                            nc.vector.tensor_tensor(
                                tgt[:].rearrange("p (k o) -> p k o", o=21),
                                s_g[g][:].rearrange("p (k o) -> p k o", o=21),
                                phi[g][:].rearrange("p (k o) -> p k o", o=1)
                                      .broadcast_to([8, 43, 21]),
                                MULT)
                            if it == 2:
                                nc.sync.dma_start(o_out[g * 8 : g * 8 + 8, :], tgt[:])
                        if it < 2:
                            for g in range(2):
                                rpp = rpsum.tile([128, KO], F32, tag="rep")
                                nc.tensor.matmul(
                                    rpp[:, 0:512], t_selr[:],
                                    out_bf[g][:, 0:512], start=True, stop=True)
                                nc.tensor.matmul(
                                    rpp[:, 512:KO], t_selr[:],
                                    out_bf[g][:, 512:KO], start=True, stop=True)
                                nc.scalar.activation(out_rep[g][:], rpp[:], ACTF.Copy)
                                for j in range(NG):
                                    arg = scr.tile([128, GT * 43], F16, tag="arg",
                                                   name="arg", bufs=2)
                                    for tj in range(GT):
                                        t = j * GT + tj
                                        ap = scr.tile([128, KO], BF16, tag="ap", bufs=6)
                                        aeng = nc.vector if (t & 3) == 0 else nc.gpsimd
                                        aeng.tensor_tensor(
                                            ap[:], P_t(g, t), out_rep[g][:], MULT)
                                        with nc.allow_low_precision("logit delta fp16"):
                                            nc.vector.tensor_reduce(
                                                arg[:, tj * 43 : tj * 43 + 43],
                                                ap[:].rearrange("p (k o) -> p k o", o=21),
                                                AXX, ADD)
                                    nc.vector.tensor_tensor(
                                        L[g][j][:], L[g][j][:], arg[:], ADD)
    _spill_extra_waits(nc)
    return nc


_CACHED = {}


def _get_bass():
    if "nc" not in _CACHED:
        _CACHED["nc"] = _build_bass()
    return _CACHED["nc"]


def kernel(**inputs):
    from concourse.bass_utils import run_bass_kernel_spmd

    d = {k: np.asarray(v) for k, v in inputs.items()}
    shared = _prep_shared(d)
    x = np.asarray(d["x"], np.float32)

    nc = _get_bass()
    in_maps = []
    for c in range(NCORES):
        m = dict(shared)
        m["xcol"] = _bf(_im2col(x[c]))
        in_maps.append(m)

    import os
    trace = bool(os.environ.get("DCAPS_TRACE"))
    res = run_bass_kernel_spmd(
        nc, in_maps, core_ids=list(range(NCORES)), trace=trace)
    _CACHED["last_results"] = res
    _CACHED["last_in_maps"] = in_maps

    out = np.empty((NCORES, 4, 4, N_CLASSES, 21), np.float32)
    for c in range(NCORES):
        r = np.asarray(res.results[c]["out"])  # (16, 903)
        for gy in range(4):
            for gx in range(4):
                cell = (gy >> 1) * 8 + (gy & 1) * 4 + gx
                out[c, gy, gx] = r[cell].reshape(N_CLASSES, 21)
    return out



# revision 2
# speedup vs baseline: 1.0794x; 1.0794x over previous
"""DarkCapsuleNet on 8 Trainium2 NeuronCores.

Data-parallel over batch (B=8, one image per core). The conv+BN+LReLU
backbone runs per core on its image; BN batch statistics are combined
across cores with tiny AllReduces (per-channel [mean, E[x^2]] sums). The
capsule-routing stage is independent per (grid-cell, image), so each core
routes its own 16 cells entirely in SBUF.

Convs are direct convolutions: matmuls accumulated over kernel offsets with
input channels on the contraction dim, bf16 operands, fp32 PSUM. Priors use
a block-diagonal lhsT built on-chip with one masked DVE multiply per tile,
so the 8-wide capsule contraction still runs as full 128-wide matmuls.
"""

import numpy as np
import ml_dtypes


class _PhaseStop(Exception):
    def __init__(self, nc):
        self.nc = nc

N_CLASSES = 43
KO = N_CLASSES * 21  # 903
EPS = 1e-5
NCORES = 8

_BF16 = ml_dtypes.bfloat16


# ---------------------------------------------------------------------------
# Workaround: this walrus build accepts at most ONE sem wait on a TPB_CTRL
# Drain instruction; Tile's epilogue drain carries one wait per HW-DMA queue.
# Split the extra waits onto standalone SP nops (same engine, before the
# all-engine barrier, so semantics are unchanged).
# ---------------------------------------------------------------------------
def _install_tile_drain_fix():
    import concourse.tile as tile_mod
    import concourse.mybir as mybir
    from concourse.vector_clock import ScopedClock

    if getattr(tile_mod.TileContext, "_drain_fix_installed", False):
        return

    def _patched(self, tick_clock, wait_clock):
        drain_inst = self.nc.sync.drain()
        wait_clock.add_sem_waits(
            drain_inst.ins, ScopedClock({None: tick_clock.global_clock})
        )
        raw = drain_inst.ins
        si = getattr(raw, "sync_info", None)
        if si is not None and si.on_wait is not None and len(si.on_wait) > 1:
            waits = list(si.on_wait)
            si.on_wait = waits[-1:]
            for w in waits[:-1]:
                nop = self.nc.sync.nop(nofuse=True, hint="split_drain_wait")
                nsi = getattr(nop.ins, "sync_info", None)
                if nsi is None:
                    nop.ins.sync_info = mybir.SyncInfo(on_update=[], on_wait=[w])
                else:
                    nw = list(nsi.on_wait) if nsi.on_wait else []
                    nw.append(w)
                    nsi.on_wait = nw
        self.nc.all_engine_barrier()
        assert self.sems is not None
        popped = self.nc._tile_sem_poison_stack.pop()
        assert popped is self._sem_poison
        self.nc.clear_and_free_semaphores(list(self.sems.allocated().values()))
        self.nc.all_engine_barrier()

    tile_mod.TileContext._drain_and_barrier = _patched
    tile_mod.TileContext._drain_fix_installed = True


# ---------------------------------------------------------------------------
# Host-side layout prep
# ---------------------------------------------------------------------------
def _bf(x):
    return np.ascontiguousarray(np.asarray(x, np.float32).astype(_BF16))


def _im2col(img):
    # img (3,128,128) f32 -> (27,16384), rows (ci,ky,kx)
    xp = np.zeros((3, 130, 130), np.float32)
    xp[:, 1:129, 1:129] = img
    cols = np.empty((3, 3, 3, 128, 128), np.float32)
    for ky in range(3):
        for kx in range(3):
            cols[:, ky, kx] = xp[:, ky : ky + 128, kx : kx + 128]
    return cols.reshape(27, 16384)


def _prep_shared(d):
    c1h = np.asarray(d["c1w"], np.float32).reshape(128, 27).T.copy()
    c2h = np.asarray(d["c2w"], np.float32).transpose(2, 3, 1, 0).reshape(9, 128, 256)
    c2h = np.concatenate(list(c2h), axis=1)  # (128, 9*256)
    c3t = np.asarray(d["c3w"], np.float32).transpose(1, 2, 3, 0)  # (256,4,4,64)
    c3h = np.concatenate(
        [c3t[m * 128 : (m + 1) * 128].reshape(128, 16 * 64) for m in range(2)], axis=1
    )  # (128, 2048)
    c4h = np.asarray(d["c4w"], np.float32).transpose(1, 2, 3, 0).reshape(64, 16 * 128)
    c5h = np.asarray(d["c5w"], np.float32).transpose(1, 2, 3, 0).reshape(128, 16 * 256)

    rw = np.asarray(d["rw"], np.float32)  # (512,43,8,21)
    rt = rw.transpose(0, 2, 1, 3).reshape(512 * 8, KO)  # row = n*8+i
    # RT[t*128 + ns*8 + i] = rw[16t+ns, :, i, :]  -> same as rt row (16t+ns)*8+i
    # rt rows are already (n,i) with n major: n*8+i = (16t+ns)*8+i = t*128+ns*8+i ✓

    gb = np.zeros((128, 14), np.float32)
    gb[:, 0] = d["g1"]; gb[:, 1] = d["b1"]
    gb[:, 2] = d["g2"][:128]; gb[:, 3] = d["b2"][:128]
    gb[:, 4] = d["g2"][128:]; gb[:, 5] = d["b2"][128:]
    gb[:64, 6] = d["g3"]; gb[:64, 7] = d["b3"]
    gb[:, 8] = d["g4"]; gb[:, 9] = d["b4"]
    gb[:, 10] = d["g5"][:128]; gb[:, 11] = d["b5"][:128]
    gb[:, 12] = d["g5"][128:]; gb[:, 13] = d["b5"][128:]

    mask = np.zeros((128, 128), np.float32)
    for p in range(128):
        mask[p, (p >> 3) * 8 : (p >> 3) * 8 + 8] = 1.0
    selb = np.zeros((128, 8), np.float32)
    for p in range(128):
        selb[p, p & 7] = 1.0
    selr = np.zeros((8, 128), np.float32)  # [b, ns*8 + b]
    for ns in range(16):
        for b in range(8):
            selr[b, ns * 8 + b] = 1.0
    return dict(
        c1wT=_bf(c1h), c2wT=_bf(c2h), c3wT=_bf(c3h), c4wT=_bf(c4h), c5wT=_bf(c5h),
        RT=_bf(rt), gb=gb, MASK=_bf(mask), SELB=_bf(selb), SELB43=_bf(selb / 43.0),
        SELR=_bf(selr),
    )


# ---------------------------------------------------------------------------
# Bass program (identical on every core)
# ---------------------------------------------------------------------------
def _spill_extra_waits(nc):
    """This walrus codegen accepts at most one semaphore wait per TPB
    instruction. Tile can attach several. Move the extras onto fresh NoOp
    instructions inserted just before the owner on the same engine."""
    import concourse.mybir as mybir

    uid = [0]
    for f in nc.m.functions:
        for bb in f.blocks:
            il = bb.instructions
            out = []
            changed = False
            for inst in il:
                si = getattr(inst, "sync_info", None)
                waits = list(si.on_wait) if si is not None and si.on_wait else []
                if len(waits) > 1:
                    for w in waits[:-1]:
                        uid[0] += 1
                        nop = mybir.InstNoOp(name=f"waitspill-{uid[0]}", ins=[], outs=[])
                        nop.engine = inst.engine
                        nop.sync_info = mybir.SyncInfo(on_update=[], on_wait=[w])
                        out.append(nop)
                    si.on_wait = waits[-1:]
                    changed = True
                out.append(inst)
            if changed:
                bb.instructions = out


def _build_bass(phase_limit=99):
    import concourse.bass as bass
    import concourse.mybir as mybir
    from concourse import tile

    _install_tile_drain_fix()

    F32 = mybir.dt.float32
    BF16 = mybir.dt.bfloat16
    F16 = mybir.dt.float16
    ADD = mybir.AluOpType.add
    MULT = mybir.AluOpType.mult
    SUB = mybir.AluOpType.subtract
    ACTF = mybir.ActivationFunctionType
    AXX = mybir.AxisListType.X

    nc = bass.Bass(num_devices=NCORES)
    dp = nc.declare_dram_parameter
    i_xcol = dp("xcol", [27, 16384], BF16, isOutput=False)
    i_c1 = dp("c1wT", [27, 128], BF16, isOutput=False)
    i_c2 = dp("c2wT", [128, 2304], BF16, isOutput=False)
    i_c3 = dp("c3wT", [128, 2048], BF16, isOutput=False)
    i_c4 = dp("c4wT", [64, 2048], BF16, isOutput=False)
    i_c5 = dp("c5wT", [128, 4096], BF16, isOutput=False)
    i_rt = dp("RT", [4096, KO], BF16, isOutput=False)
    i_gb = dp("gb", [128, 14], F32, isOutput=False)
    i_mask = dp("MASK", [128, 128], BF16, isOutput=False)
    i_selb = dp("SELB", [128, 8], BF16, isOutput=False)
    i_selb43 = dp("SELB43", [128, 8], BF16, isOutput=False)
    i_selr = dp("SELR", [8, 128], BF16, isOutput=False)
    o_out = dp("out", [16, KO], F32, isOutput=True)


    with tile.TileContext(nc) as tc:
        with tc.tile_pool(name="const", bufs=1) as const, \
             tc.tile_pool(name="dram", bufs=1, space="DRAM") as dram:
            t_gb = const.tile([128, 14], F32)
            t_mask = const.tile([128, 128], BF16)
            t_selb = const.tile([128, 8], BF16)
            t_selb43 = const.tile([128, 8], BF16)
            t_selr = const.tile([8, 128], BF16)
            h5 = [const.tile([128, 256], BF16, tag=f"h5_{m}", name=f"h5_{m}") for m in range(2)]
            t_st6 = const.tile([128, 32 * 6], F32)
            t_mv = const.tile([128, 4], F32)
            t_ab = const.tile([128, 4], F32)
            t_sc = const.tile([128, 2], F32)
            for t, i in [(t_gb, i_gb), (t_mask, i_mask), (t_selb, i_selb),
                         (t_selb43, i_selb43), (t_selr, i_selr)]:
                nc.sync.dma_start(t[:], i[:])

            ar_in = [dram.tile([128, 4], F32, tag=f"ari{i}", name=f"ari{i}") for i in range(5)]
            ar_out = [dram.tile([8, 512], F32, tag=f"aro{i}", name=f"aro{i}") for i in range(5)]
            t_ag = const.tile([128, 32], F32)

            def bn_allreduce(layer, nch_tiles, npart):
                """t_mv holds per-core [m0,v0,m1,v1]; leaves [a0,b0,a1,b1] in t_ab."""
                for mt in range(nch_tiles):
                    m = t_mv[:npart, 2 * mt : 2 * mt + 1]
                    v = t_mv[:npart, 2 * mt + 1 : 2 * mt + 2]
                    s1 = t_sc[:npart, 0:1]
                    nc.scalar.activation(s1, m, ACTF.Square)
                    nc.vector.tensor_tensor(v, v, s1, ADD)  # v := E[x^2] local
                nc.sync.dma_start(ar_in[layer][:], t_mv[:])
                nc.gpsimd.collective_compute(
                    "AllGather", mybir.AluOpType.bypass,
                    ins=[ar_in[layer][:]], outs=[ar_out[layer][:]],
                    replica_groups=[list(range(NCORES))],
                )
                nc.sync.dma_start(
                    t_ag[:].rearrange("p (g c) -> p g c", c=4),
                    ar_out[layer][:].rearrange("g (p c) -> p g c", c=4),
                )
                nc.vector.tensor_reduce(
                    t_mv[:, 0:4], t_ag[:].rearrange("p (g c) -> p c g", c=4),
                    AXX, ADD)
                for mt in range(nch_tiles):
                    m = t_mv[:npart, 2 * mt : 2 * mt + 1]
                    q = t_mv[:npart, 2 * mt + 1 : 2 * mt + 2]
                    a = t_ab[:npart, 2 * mt : 2 * mt + 1]
                    b = t_ab[:npart, 2 * mt + 1 : 2 * mt + 2]
                    s1 = t_sc[:npart, 0:1]
                    nc.vector.tensor_scalar_mul(m, m, 1.0 / NCORES)
                    nc.vector.tensor_scalar_mul(q, q, 1.0 / NCORES)
                    nc.scalar.activation(s1, m, ACTF.Square)
                    nc.vector.tensor_tensor(q, q, s1, SUB)       # gvar
                    nc.vector.tensor_scalar_add(q, q, EPS)
                    nc.vector.reciprocal(s1, q)
                    nc.scalar.activation(s1, s1, ACTF.Sqrt)      # rsqrt(var+eps)
                    gcol = (0, 2, 6, 8, 10)[layer] + 2 * mt
                    nc.vector.tensor_tensor(a, t_gb[:npart, gcol : gcol + 1], s1, MULT)
                    nc.vector.tensor_tensor(s1, a, m, MULT)
                    nc.vector.tensor_tensor(b, t_gb[:npart, gcol + 1 : gcol + 2], s1, SUB)

            def lrelu_apply(view, scale, bias):
                nc.scalar.activation(view, view, ACTF.Prelu,
                                     bias=bias, scale=scale, alpha=0.1)

            # ================= conv backbone =================
            with tc.tile_pool(name="wpool", bufs=1) as wp, \
                 tc.tile_pool(name="xpool", bufs=1) as xp, \
                 tc.tile_pool(name="acts", bufs=1) as acts, \
                 tc.tile_pool(name="cpsum", bufs=4, space="PSUM") as cpsum:
                t_c2 = wp.tile([128, 2304], BF16)
                t_c3 = wp.tile([128, 2048], BF16)
                t_c4 = wp.tile([64, 2048], BF16)
                t_c5 = wp.tile([128, 4096], BF16)
                t_c1 = xp.tile([27, 128], BF16)
                t_xcol = xp.tile([27, 16384], BF16)
                nc.sync.dma_start(t_c1[:], i_c1[:])
                for ch in range(4):
                    nc.sync.dma_start(t_xcol[:, ch * 4096 : (ch + 1) * 4096],
                                      i_xcol[:, ch * 4096 : (ch + 1) * 4096])

                h1 = acts.tile([128, 130 * 130], BF16)
                h2 = [acts.tile([128, 130 * 130], BF16, tag=f"h2_{m}", name=f"h2_{m}") for m in range(2)]
                h3 = acts.tile([64, 66 * 66], BF16)
                h4 = acts.tile([128, 34 * 34], BF16)

                def zero_border(tile_ap, H):
                    v = tile_ap.rearrange("p (a b) -> p a b", b=H)
                    nc.gpsimd.memset(v[:, 0:1, :], 0.0)
                    nc.gpsimd.memset(v[:, H - 1 : H, :], 0.0)
                    nc.gpsimd.memset(v[:, 1 : H - 1, 0:1], 0.0)
                    nc.gpsimd.memset(v[:, 1 : H - 1, H - 1 : H], 0.0)

                zero_border(h1[:], 130)
                zero_border(h2[0][:], 130)
                zero_border(h2[1][:], 130)
                zero_border(h3[:], 66)
                zero_border(h4[:], 34)

                # ---- conv1 ----
                for nt in range(32):
                    ps = cpsum.tile([128, 512], F32, tag="cps")
                    nc.tensor.matmul(ps[:], t_c1[:],
                                     t_xcol[:, nt * 512 : (nt + 1) * 512],
                                     start=True, stop=True)
                    intr = h1[:].rearrange("p (a b) -> p a b", b=130)[
                        :, 1 + nt * 4 : 5 + nt * 4, 1:129]
                    nc.scalar.activation(
                        intr, ps[:].rearrange("p (a b) -> p a b", b=128), ACTF.Copy)
                    nc.vector.bn_stats(t_st6[:, nt * 6 : nt * 6 + 6], ps[:])
                for t, i in [(t_c2, i_c2), (t_c3, i_c3), (t_c4, i_c4),
                             (t_c5, i_c5)]:
                    nc.sync.dma_start(t[:], i[:])
                nc.vector.bn_aggr(t_mv[:, 0:2],
                                  t_st6[:].rearrange("p (g s) -> p g s", s=6))
                bn_allreduce(0, 1, 128)
                h1v = h1[:].rearrange("p (a b) -> p a b", b=130)
                for c4_ in range(4):
                    lrelu_apply(h1v[:, 1 + 32 * c4_ : 33 + 32 * c4_, 1:129],
                                t_ab[:, 0:1], t_ab[:, 1:2])

                # ---- conv2 ----
                if phase_limit < 2:
                    raise _PhaseStop(nc)
                for m in range(2):
                    for nt in range(32):
                        ps = cpsum.tile([128, 512], F32, tag="cps")
                        for off in range(9):
                            ky, kx = off // 3, off % 3
                            rhs = h1v[:, ky + nt * 4 : ky + nt * 4 + 4, kx : kx + 128]
                            nc.tensor.matmul(
                                ps[:],
                                t_c2[:, off * 256 + m * 128 : off * 256 + m * 128 + 128],
                                rhs, start=(off == 0), stop=(off == 8))
                        intr = h2[m][:].rearrange("p (a b) -> p a b", b=130)[
                            :, 1 + nt * 4 : 5 + nt * 4, 1:129]
                        nc.scalar.activation(
                            intr, ps[:].rearrange("p (a b) -> p a b", b=128), ACTF.Copy)
                        nc.vector.bn_stats(t_st6[:, nt * 6 : nt * 6 + 6], ps[:])
                    nc.vector.bn_aggr(t_mv[:, 2 * m : 2 * m + 2],
                                      t_st6[:].rearrange("p (g s) -> p g s", s=6))
                bn_allreduce(1, 2, 128)
                h2v = [h2[m][:].rearrange("p (a b) -> p a b", b=130) for m in range(2)]
                for m in range(2):
                    for c4_ in range(4):
                        lrelu_apply(h2v[m][:, 1 + 32 * c4_ : 33 + 32 * c4_, 1:129],
                                    t_ab[:, 2 * m : 2 * m + 1],
                                    t_ab[:, 2 * m + 1 : 2 * m + 2])

                # ---- conv3 ----
                if phase_limit < 3:
                    raise _PhaseStop(nc)
                for nt in range(8):
                    ps = cpsum.tile([128, 512], F32, tag="cps")
                    first = True
                    for m in range(2):
                        for off in range(16):
                            ky, kx = off // 4, off % 4
                            rhs = h2v[m][:, ky + nt * 16 : ky + nt * 16 + 15 : 2,
                                         kx : kx + 127 : 2]
                            nc.tensor.matmul(
                                ps[:64, :],
                                t_c3[:, (m * 16 + off) * 64 : (m * 16 + off) * 64 + 64],
                                rhs, start=first, stop=(m == 1 and off == 15))
                            first = False
                    intr = h3[:].rearrange("p (a b) -> p a b", b=66)[
                        :, 1 + nt * 8 : 9 + nt * 8, 1:65]
                    nc.scalar.activation(
                        intr, ps[:64, :].rearrange("p (a b) -> p a b", b=64), ACTF.Copy)
                    nc.vector.bn_stats(t_st6[:64, nt * 6 : nt * 6 + 6], ps[:64, :])
                nc.vector.bn_aggr(
                    t_mv[:64, 0:2],
                    t_st6[:64, : 8 * 6].rearrange("p (g s) -> p g s", s=6))
                bn_allreduce(2, 1, 64)
                h3v = h3[:].rearrange("p (a b) -> p a b", b=66)
                lrelu_apply(h3v[:, 1:65, 1:65], t_ab[:64, 0:1], t_ab[:64, 1:2])

                # ---- conv4 ----
                if phase_limit < 4:
                    raise _PhaseStop(nc)
                for nt in range(2):
                    ps = cpsum.tile([128, 512], F32, tag="cps")
                    for off in range(16):
                        ky, kx = off // 4, off % 4
                        rhs = h3v[:, ky + nt * 32 : ky + nt * 32 + 31 : 2, kx : kx + 63 : 2]
                        nc.tensor.matmul(ps[:], t_c4[:, off * 128 : off * 128 + 128],
                                         rhs, start=(off == 0), stop=(off == 15))
                    intr = h4[:].rearrange("p (a b) -> p a b", b=34)[
                        :, 1 + nt * 16 : 17 + nt * 16, 1:33]
                    nc.scalar.activation(
                        intr, ps[:].rearrange("p (a b) -> p a b", b=32), ACTF.Copy)
                    nc.vector.bn_stats(t_st6[:, nt * 6 : nt * 6 + 6], ps[:])
                nc.vector.bn_aggr(
                    t_mv[:, 0:2], t_st6[:, :12].rearrange("p (g s) -> p g s", s=6))
                bn_allreduce(3, 1, 128)
                h4v = h4[:].rearrange("p (a b) -> p a b", b=34)
                lrelu_apply(h4v[:, 1:33, 1:33], t_ab[:, 0:1], t_ab[:, 1:2])

                # ---- conv5 ----
                if phase_limit < 5:
                    raise _PhaseStop(nc)
                for m in range(2):
                    ps = cpsum.tile([128, 512], F32, tag="cps")
                    first = True
                    for off in range(16):
                        ky, kx = off // 4, off % 4
                        rhs = h4v[:, ky : ky + 31 : 2, kx : kx + 31 : 2]
                        nc.tensor.matmul(
                            ps[:, 0:256],
                            t_c5[:, off * 256 + m * 128 : off * 256 + m * 128 + 128],
                            rhs, start=first, stop=(off == 15))
                        first = False
                    nc.scalar.activation(h5[m][:], ps[:, 0:256], ACTF.Copy)
                    nc.vector.bn_stats(t_st6[:, m * 6 : m * 6 + 6], ps[:, 0:256])
                for m in range(2):
                    nc.vector.bn_aggr(
                        t_mv[:, 2 * m : 2 * m + 2],
                        t_st6[:, m * 6 : m * 6 + 6].rearrange("p (g s) -> p g s", s=6))
                bn_allreduce(4, 2, 128)
                for m in range(2):
                    lrelu_apply(h5[m][:], t_ab[:, 2 * m : 2 * m + 1],
                                t_ab[:, 2 * m + 1 : 2 * m + 2])

            if phase_limit < 6:
                raise _PhaseStop(nc)
            # ================= priors =================
            with tc.tile_pool(name="pri", bufs=1) as pri, \
                 tc.tile_pool(name="route", bufs=1) as rp, \
                 tc.tile_pool(name="scr", bufs=4) as scr:
                P = [[pri.tile([128, 8 * KO], BF16, tag=f"P{g}_{j}", name=f"P{g}_{j}")
                      for j in range(4)] for g in range(2)]

                def P_t(g, t):
                    j, tj = t // 8, t % 8
                    return P[g][j][:, tj * KO : tj * KO + KO]
                with tc.tile_pool(name="ppsum", bufs=3, space="PSUM") as ppsum:
                    for t in range(32):
                        h = t >> 3
                        w = (t >> 1) & 3
                        mblk = t & 1
                        rt_t = scr.tile([128, KO], BF16, tag="rt", bufs=8)
                        nc.sync.dma_start(rt_t[:], i_rt[t * 128 : (t + 1) * 128, :])
                        hb = h5[mblk][:].rearrange(
                            "p (hh gy gx ww) -> p hh gy gx ww",
                            hh=4, gy=4, gx=4)
                        for g in range(2):
                            g8 = scr.tile([128, 8], BF16, tag="g8")
                            src = hb[:, h : h + 1, 2 * g : 2 * g + 2, :, w : w + 1]
                            # (p,1,2,4,1) -> (p,2,4)
                            src = src.rearrange("p a b d e -> p (a b) (d e)")
                            nc.gpsimd.tensor_copy(
                                g8[:].rearrange("p (b d) -> p b d", b=2), src)
                            lt = scr.tile([128, 128], BF16, tag="lt")
                            nc.vector.tensor_tensor(
                                lt[:].rearrange("p (n b) -> p n b", b=8),
                                g8[:].rearrange("p (o e) -> p o e", o=1)
                                    .broadcast_to([128, 16, 8]),
                                t_mask[:].rearrange("p (n b) -> p n b", b=8),
                                MULT)
                            pp = ppsum.tile([128, KO], F32, tag="pps")
                            nc.tensor.matmul(pp[:, 0:512], lt[:], rt_t[:, 0:512],
                                             start=True, stop=True)
                            nc.tensor.matmul(pp[:, 512:KO], lt[:], rt_t[:, 512:KO],
                                             start=True, stop=True)
                            if (t & 3) == 0:
                                nc.vector.tensor_copy(P_t(g, t), pp[:])
                            else:
                                nc.scalar.activation(P_t(g, t), pp[:], ACTF.Copy)

                # ================= routing =================
                if phase_limit < 7:
                    raise _PhaseStop(nc)
                NG = 4   # tile-groups per cell-group (8 tiles each)
                GT = 8
                L = [[rp.tile([128, GT * 43], F16, tag=f"L{g}_{j}", name=f"L{g}_{j}")
                      for j in range(NG)] for g in range(2)]
                PR = [[rp.tile([128, GT * 43], BF16, tag=f"PR{g}_{j}", name=f"PR{g}_{j}")
                       for j in range(NG)] for g in range(2)]
                s_g = [rp.tile([8, KO], F32, tag=f"s_g{g}", name=f"s_g{g}") for g in range(2)]
                sq = [rp.tile([8, KO], F32, tag=f"sq{g}", name=f"sq{g}") for g in range(2)]
                sn = [rp.tile([8, 43], F32, tag=f"sn{g}", name=f"sn{g}") for g in range(2)]
                den = [rp.tile([8, 43], F32, tag=f"den{g}", name=f"den{g}") for g in range(2)]
                phi = [rp.tile([8, 43], F32, tag=f"phi{g}", name=f"phi{g}") for g in range(2)]
                out_f = [rp.tile([8, KO], F32, tag=f"of{g}", name=f"of{g}") for g in range(2)]
                out_bf = [rp.tile([8, KO], BF16, tag=f"ob{g}", name=f"ob{g}") for g in range(2)]
                out_rep = [rp.tile([128, KO], BF16, tag=f"orep{g}", name=f"orep{g}") for g in range(2)]
                for g in range(2):
                    for j in range(NG):
                        nc.vector.memset(L[g][j][:], 0.0)

                with tc.tile_pool(name="rpsum", bufs=2, space="PSUM") as rpsum:
                    for it in range(3):
                        for g in range(2):
                            if it > 0:
                                for j in range(NG):
                                    e8 = scr.tile([128, GT * 43], F16, tag="e8")
                                    nc.scalar.activation(e8[:], L[g][j][:], ACTF.Exp)
                                    r8 = scr.tile([128, GT], F32, tag="r8")
                                    nc.vector.tensor_reduce(
                                        r8[:], e8[:].rearrange("p (t k) -> p t k", k=43),
                                        AXX, ADD)
                                    nc.vector.reciprocal(r8[:], r8[:])
                                    nc.vector.tensor_tensor(
                                        PR[g][j][:].rearrange("p (t k) -> p t k", k=43),
                                        e8[:].rearrange("p (t k) -> p t k", k=43),
                                        r8[:].rearrange("p (t k) -> p t k", k=1)
                                            .broadcast_to([128, GT, 43]),
                                        MULT)
                            sp = rpsum.tile([8, KO], F32, tag="sps")
                            for t in range(32):
                                j, tj = t // GT, t % GT
                                if it == 0:
                                    rhs_t = P_t(g, t)
                                    lhs = t_selb43
                                else:
                                    tm = scr.tile([128, KO], BF16, tag="tm", bufs=6)
                                    teng = nc.gpsimd if (t & 3) == 3 else nc.vector
                                    teng.tensor_tensor(
                                        tm[:].rearrange("p (k o) -> p k o", o=21),
                                        P_t(g, t).rearrange("p (k o) -> p k o", o=21),
                                        PR[g][j][:, tj * 43 : tj * 43 + 43]
                                        .rearrange("p (k o) -> p k o", o=1)
                                        .broadcast_to([128, 43, 21]),
                                        MULT)
                                    rhs_t = tm[:]
                                    lhs = t_selb
                                nc.tensor.matmul(sp[:, 0:512], lhs[:], rhs_t[:, 0:512],
                                                 start=(t == 0), stop=(t == 31))
                                nc.tensor.matmul(sp[:, 512:KO], lhs[:], rhs_t[:, 512:KO],
                                                 start=(t == 0), stop=(t == 31))
                            nc.scalar.activation(s_g[g][:], sp[:], ACTF.Copy)
                        # squash: out = s * sqrt(sn)/(1+sn)
                        for g in range(2):
                            nc.scalar.activation(sq[g][:], s_g[g][:], ACTF.Square)
                            nc.vector.tensor_reduce(
                                sn[g][:], sq[g][:].rearrange("p (k o) -> p k o", o=21),
                                AXX, ADD)
                            nc.vector.tensor_scalar_add(den[g][:], sn[g][:], 1.0)
                            nc.vector.reciprocal(den[g][:], den[g][:])
                            nc.scalar.activation(phi[g][:], sn[g][:], ACTF.Sqrt)
                            nc.vector.tensor_tensor(phi[g][:], phi[g][:], den[g][:], MULT)
                            tgt = out_f[g] if it == 2 else out_bf[g]
                            nc.vector.tensor_tensor(
                                tgt[:].rearrange("p (k o) -> p k o", o=21),
                                s_g[g][:].rearrange("p (k o) -> p k o", o=21),
                                phi[g][:].rearrange("p (k o) -> p k o", o=1)
                                      .broadcast_to([8, 43, 21]),
                                MULT)
                            if it == 2:
                                nc.sync.dma_start(o_out[g * 8 : g * 8 + 8, :], tgt[:])
                        if it < 2:
                            for g in range(2):
                                rpp = rpsum.tile([128, KO], F32, tag="rep")
                                nc.tensor.matmul(
                                    rpp[:, 0:512], t_selr[:],
                                    out_bf[g][:, 0:512], start=True, stop=True)
                                nc.tensor.matmul(
                                    rpp[:, 512:KO], t_selr[:],
                                    out_bf[g][:, 512:KO], start=True, stop=True)
                                nc.scalar.activation(out_rep[g][:], rpp[:], ACTF.Copy)
                                for j in range(NG):
                                    arg = scr.tile([128, GT * 43], F16, tag="arg",
                                                   name="arg", bufs=2)
                                    for tj in range(GT):
                                        t = j * GT + tj
                                        ap = scr.tile([128, KO], BF16, tag="ap", bufs=6)
                                        aeng = nc.vector if (t & 3) == 0 else nc.gpsimd
                                        aeng.tensor_tensor(
                                            ap[:], P_t(g, t), out_rep[g][:], MULT)
                                        with nc.allow_low_precision("logit delta fp16"):
                                            nc.vector.tensor_reduce(
                                                arg[:, tj * 43 : tj * 43 + 43],
                                                ap[:].rearrange("p (k o) -> p k o", o=21),
                                                AXX, ADD)
                                    nc.vector.tensor_tensor(
                                        L[g][j][:], L[g][j][:], arg[:], ADD)
    _spill_extra_waits(nc)
    return nc


_CACHED = {}


def _get_bass():
    if "nc" not in _CACHED:
        _CACHED["nc"] = _build_bass()
    return _CACHED["nc"]


def kernel(**inputs):
    from concourse.bass_utils import run_bass_kernel_spmd

    d = {k: np.asarray(v) for k, v in inputs.items()}
    shared = _prep_shared(d)
    x = np.asarray(d["x"], np.float32)

    nc = _get_bass()
    in_maps = []
    for c in range(NCORES):
        m = dict(shared)
        m["xcol"] = _bf(_im2col(x[c]))
        in_maps.append(m)

    import os
    trace = bool(os.environ.get("DCAPS_TRACE"))
    res = run_bass_kernel_spmd(
        nc, in_maps, core_ids=list(range(NCORES)), trace=trace)
    _CACHED["last_results"] = res
    _CACHED["last_in_maps"] = in_maps

    out = np.empty((NCORES, 4, 4, N_CLASSES, 21), np.float32)
    for c in range(NCORES):
        r = np.asarray(res.results[c]["out"])  # (16, 903)
        for gy in range(4):
            for gx in range(4):
                cell = (gy >> 1) * 8 + (gy & 1) * 4 + gx
                out[c, gy, gx] = r[cell].reshape(N_CLASSES, 21)
    return out



# revision 13
# speedup vs baseline: 1.1276x; 1.0447x over previous
"""DarkCapsuleNet on 8 Trainium2 NeuronCores.

Data-parallel over batch (B=8, one image per core). The conv+BN+LReLU
backbone runs per core on its image; BN batch statistics are combined
across cores with tiny AllReduces (per-channel [mean, E[x^2]] sums). The
capsule-routing stage is independent per (grid-cell, image), so each core
routes its own 16 cells entirely in SBUF.

Convs are direct convolutions: matmuls accumulated over kernel offsets with
input channels on the contraction dim, bf16 operands, fp32 PSUM. Priors use
a block-diagonal lhsT built on-chip with one masked DVE multiply per tile,
so the 8-wide capsule contraction still runs as full 128-wide matmuls.
"""

import numpy as np
import ml_dtypes


class _PhaseStop(Exception):
    def __init__(self, nc):
        self.nc = nc

N_CLASSES = 43
KO = N_CLASSES * 21  # 903
EPS = 1e-5
NCORES = 8

_BF16 = ml_dtypes.bfloat16


# ---------------------------------------------------------------------------
# Workaround: this walrus build accepts at most ONE sem wait on a TPB_CTRL
# Drain instruction; Tile's epilogue drain carries one wait per HW-DMA queue.
# Split the extra waits onto standalone SP nops (same engine, before the
# all-engine barrier, so semantics are unchanged).
# ---------------------------------------------------------------------------
def _install_tile_drain_fix():
    import concourse.tile as tile_mod
    import concourse.mybir as mybir
    from concourse.vector_clock import ScopedClock

    if getattr(tile_mod.TileContext, "_drain_fix_installed", False):
        return

    def _patched(self, tick_clock, wait_clock):
        drain_inst = self.nc.sync.drain()
        wait_clock.add_sem_waits(
            drain_inst.ins, ScopedClock({None: tick_clock.global_clock})
        )
        raw = drain_inst.ins
        si = getattr(raw, "sync_info", None)
        if si is not None and si.on_wait is not None and len(si.on_wait) > 1:
            waits = list(si.on_wait)
            si.on_wait = waits[-1:]
            for w in waits[:-1]:
                nop = self.nc.sync.nop(nofuse=True, hint="split_drain_wait")
                nsi = getattr(nop.ins, "sync_info", None)
                if nsi is None:
                    nop.ins.sync_info = mybir.SyncInfo(on_update=[], on_wait=[w])
                else:
                    nw = list(nsi.on_wait) if nsi.on_wait else []
                    nw.append(w)
                    nsi.on_wait = nw
        self.nc.all_engine_barrier()
        assert self.sems is not None
        popped = self.nc._tile_sem_poison_stack.pop()
        assert popped is self._sem_poison
        self.nc.clear_and_free_semaphores(list(self.sems.allocated().values()))
        self.nc.all_engine_barrier()

    tile_mod.TileContext._drain_and_barrier = _patched
    tile_mod.TileContext._drain_fix_installed = True


# ---------------------------------------------------------------------------
# Host-side layout prep
# ---------------------------------------------------------------------------
def _bf(x):
    return np.ascontiguousarray(np.asarray(x, np.float32).astype(_BF16))


def _im2col(img):
    # img (3,128,128) f32 -> (27,16384), rows (ci,ky,kx)
    xp = np.zeros((3, 130, 130), np.float32)
    xp[:, 1:129, 1:129] = img
    cols = np.empty((3, 3, 3, 128, 128), np.float32)
    for ky in range(3):
        for kx in range(3):
            cols[:, ky, kx] = xp[:, ky : ky + 128, kx : kx + 128]
    return cols.reshape(27, 16384)


def _prep_shared(d):
    c1h = np.asarray(d["c1w"], np.float32).reshape(128, 27).T.copy()
    c2h = np.asarray(d["c2w"], np.float32).transpose(2, 3, 1, 0).reshape(9, 128, 256)
    c2h = np.concatenate(list(c2h), axis=1)  # (128, 9*256)
    c3t = np.asarray(d["c3w"], np.float32).transpose(1, 2, 3, 0)  # (256,4,4,64)
    c3h = np.concatenate(
        [c3t[m * 128 : (m + 1) * 128].reshape(128, 16 * 64) for m in range(2)], axis=1
    )  # (128, 2048)
    c4h = np.asarray(d["c4w"], np.float32).transpose(1, 2, 3, 0).reshape(64, 16 * 128)
    c5h = np.asarray(d["c5w"], np.float32).transpose(1, 2, 3, 0).reshape(128, 16 * 256)

    rw = np.asarray(d["rw"], np.float32)  # (512,43,8,21)
    # o-major columns: col = o*43 + k (so k is innermost => packed 2-byte
    # innermost dims everywhere in routing => DVE 2x perf mode)
    rt = rw.transpose(0, 2, 3, 1).reshape(512 * 8, KO)  # row = n*8+i, col = o*43+k

    gb = np.zeros((128, 14), np.float32)
    gb[:, 0] = d["g1"]; gb[:, 1] = d["b1"]
    gb[:, 2] = d["g2"][:128]; gb[:, 3] = d["b2"][:128]
    gb[:, 4] = d["g2"][128:]; gb[:, 5] = d["b2"][128:]
    gb[:64, 6] = d["g3"]; gb[:64, 7] = d["b3"]
    gb[:, 8] = d["g4"]; gb[:, 9] = d["b4"]
    gb[:, 10] = d["g5"][:128]; gb[:, 11] = d["b5"][:128]
    gb[:, 12] = d["g5"][128:]; gb[:, 13] = d["b5"][128:]

    mask = np.zeros((128, 128), np.float32)
    for p in range(128):
        mask[p, (p >> 3) * 8 : (p >> 3) * 8 + 8] = 1.0
    selb = np.zeros((128, 8), np.float32)
    for p in range(128):
        selb[p, p & 7] = 1.0
    selr = np.zeros((8, 128), np.float32)  # [b, ns*8 + b]
    for ns in range(16):
        for b in range(8):
            selr[b, ns * 8 + b] = 1.0
    return dict(
        c1wT=_bf(c1h), c2wT=_bf(c2h), c3wT=_bf(c3h), c4wT=_bf(c4h), c5wT=_bf(c5h),
        RT=_bf(rt), gb=gb, MASK=_bf(mask), SELB=_bf(selb), SELB43=_bf(selb / 43.0),
        SELR=_bf(selr),
    )


# ---------------------------------------------------------------------------
# Bass program (identical on every core)
# ---------------------------------------------------------------------------
def _spill_extra_waits(nc):
    """This walrus codegen accepts at most one semaphore wait per TPB
    instruction. Tile can attach several. Move the extras onto fresh NoOp
    instructions inserted just before the owner on the same engine."""
    import concourse.mybir as mybir

    uid = [0]
    for f in nc.m.functions:
        for bb in f.blocks:
            il = bb.instructions
            out = []
            changed = False
            for inst in il:
                si = getattr(inst, "sync_info", None)
                waits = list(si.on_wait) if si is not None and si.on_wait else []
                if len(waits) > 1:
                    for w in waits[:-1]:
                        uid[0] += 1
                        nop = mybir.InstNoOp(name=f"waitspill-{uid[0]}", ins=[], outs=[])
                        nop.engine = inst.engine
                        nop.sync_info = mybir.SyncInfo(on_update=[], on_wait=[w])
                        out.append(nop)
                    si.on_wait = waits[-1:]
                    changed = True
                out.append(inst)
            if changed:
                bb.instructions = out


def _build_bass(phase_limit=99):
    import concourse.bass as bass
    import concourse.mybir as mybir
    from concourse import tile

    _install_tile_drain_fix()

    F32 = mybir.dt.float32
    BF16 = mybir.dt.bfloat16
    F16 = mybir.dt.float16
    ADD = mybir.AluOpType.add
    MULT = mybir.AluOpType.mult
    SUB = mybir.AluOpType.subtract
    ACTF = mybir.ActivationFunctionType
    AXX = mybir.AxisListType.X

    nc = bass.Bass(num_devices=NCORES)
    dp = nc.declare_dram_parameter
    i_xcol = dp("xcol", [27, 16384], BF16, isOutput=False)
    i_c1 = dp("c1wT", [27, 128], BF16, isOutput=False)
    i_c2 = dp("c2wT", [128, 2304], BF16, isOutput=False)
    i_c3 = dp("c3wT", [128, 2048], BF16, isOutput=False)
    i_c4 = dp("c4wT", [64, 2048], BF16, isOutput=False)
    i_c5 = dp("c5wT", [128, 4096], BF16, isOutput=False)
    i_rt = dp("RT", [4096, KO], BF16, isOutput=False)
    i_gb = dp("gb", [128, 14], F32, isOutput=False)
    i_mask = dp("MASK", [128, 128], BF16, isOutput=False)
    i_selb = dp("SELB", [128, 8], BF16, isOutput=False)
    i_selb43 = dp("SELB43", [128, 8], BF16, isOutput=False)
    i_selr = dp("SELR", [8, 128], BF16, isOutput=False)
    o_out = dp("out", [16, KO], F32, isOutput=True)


    with tile.TileContext(nc) as tc:
        with tc.tile_pool(name="const", bufs=1) as const, \
             tc.tile_pool(name="dram", bufs=1, space="DRAM") as dram:
            t_gb = const.tile([128, 14], F32)
            t_mask = const.tile([128, 128], BF16)
            t_selb = const.tile([128, 8], BF16)
            t_selb43 = const.tile([128, 8], BF16)
            t_selr = const.tile([8, 128], BF16)
            h5 = [const.tile([128, 256], BF16, tag=f"h5_{m}", name=f"h5_{m}") for m in range(2)]
            t_st6 = const.tile([128, 32 * 6], F32)
            t_mv = const.tile([128, 4], F32)
            t_ab = const.tile([128, 4], F32)
            t_sc = const.tile([128, 2], F32)
            for t, i in [(t_gb, i_gb), (t_mask, i_mask), (t_selb, i_selb),
                         (t_selb43, i_selb43), (t_selr, i_selr)]:
                nc.sync.dma_start(t[:], i[:])

            ar_in = [dram.tile([128, 4], F32, tag=f"ari{i}", name=f"ari{i}") for i in range(5)]
            ar_out = [dram.tile([8, 512], F32, tag=f"aro{i}", name=f"aro{i}") for i in range(5)]
            t_ag = const.tile([128, 32], F32)

            def bn_allreduce(layer, nch_tiles, npart):
                """t_mv holds per-core [m0,v0,m1,v1]; leaves [a0,b0,a1,b1] in t_ab."""
                for mt in range(nch_tiles):
                    m = t_mv[:npart, 2 * mt : 2 * mt + 1]
                    v = t_mv[:npart, 2 * mt + 1 : 2 * mt + 2]
                    s1 = t_sc[:npart, 0:1]
                    nc.scalar.activation(s1, m, ACTF.Square)
                    nc.vector.tensor_tensor(v, v, s1, ADD)  # v := E[x^2] local
                nc.sync.dma_start(ar_in[layer][:], t_mv[:])
                nc.gpsimd.collective_compute(
                    "AllGather", mybir.AluOpType.bypass,
                    ins=[ar_in[layer][:]], outs=[ar_out[layer][:]],
                    replica_groups=[list(range(NCORES))],
                )
                nc.sync.dma_start(
                    t_ag[:].rearrange("p (g c) -> p g c", c=4),
                    ar_out[layer][:].rearrange("g (p c) -> p g c", c=4),
                )
                nc.vector.tensor_reduce(
                    t_mv[:, 0:4], t_ag[:].rearrange("p (g c) -> p c g", c=4),
                    AXX, ADD)
                for mt in range(nch_tiles):
                    m = t_mv[:npart, 2 * mt : 2 * mt + 1]
                    q = t_mv[:npart, 2 * mt + 1 : 2 * mt + 2]
                    a = t_ab[:npart, 2 * mt : 2 * mt + 1]
                    b = t_ab[:npart, 2 * mt + 1 : 2 * mt + 2]
                    s1 = t_sc[:npart, 0:1]
                    nc.vector.tensor_scalar_mul(m, m, 1.0 / NCORES)
                    nc.vector.tensor_scalar_mul(q, q, 1.0 / NCORES)
                    nc.scalar.activation(s1, m, ACTF.Square)
                    nc.vector.tensor_tensor(q, q, s1, SUB)       # gvar
                    nc.vector.tensor_scalar_add(q, q, EPS)
                    nc.vector.reciprocal(s1, q)
                    nc.scalar.activation(s1, s1, ACTF.Sqrt)      # rsqrt(var+eps)
                    gcol = (0, 2, 6, 8, 10)[layer] + 2 * mt
                    nc.vector.tensor_tensor(a, t_gb[:npart, gcol : gcol + 1], s1, MULT)
                    nc.vector.tensor_tensor(s1, a, m, MULT)
                    nc.vector.tensor_tensor(b, t_gb[:npart, gcol + 1 : gcol + 2], s1, SUB)

            def lrelu_apply(view, scale, bias):
                nc.scalar.activation(view, view, ACTF.Prelu,
                                     bias=bias, scale=scale, alpha=0.1)

            # ================= conv backbone =================
            with tc.tile_pool(name="wpool", bufs=1) as wp, \
                 tc.tile_pool(name="xpool", bufs=1) as xp, \
                 tc.tile_pool(name="acts", bufs=1) as acts, \
                 tc.tile_pool(name="cpsum", bufs=4, space="PSUM") as cpsum:
                t_c2 = wp.tile([128, 2304], BF16)
                t_c3 = wp.tile([128, 2048], BF16)
                t_c4 = wp.tile([64, 2048], BF16)
                t_c5 = wp.tile([128, 4096], BF16)
                t_c1 = xp.tile([27, 128], BF16)
                t_xcol = xp.tile([27, 16384], BF16)
                nc.sync.dma_start(t_c1[:], i_c1[:])
                for ch in range(4):
                    nc.sync.dma_start(t_xcol[:, ch * 4096 : (ch + 1) * 4096],
                                      i_xcol[:, ch * 4096 : (ch + 1) * 4096])

                h1 = acts.tile([128, 130 * 130], BF16)
                h2 = [acts.tile([128, 130 * 130], BF16, tag=f"h2_{m}", name=f"h2_{m}") for m in range(2)]
                h3 = acts.tile([64, 66 * 66], BF16)
                h4 = acts.tile([128, 34 * 34], BF16)

                def zero_border(tile_ap, H):
                    v = tile_ap.rearrange("p (a b) -> p a b", b=H)
                    nc.gpsimd.memset(v[:, 0:1, :], 0.0)
                    nc.gpsimd.memset(v[:, H - 1 : H, :], 0.0)
                    nc.gpsimd.memset(v[:, 1 : H - 1, 0:1], 0.0)
                    nc.gpsimd.memset(v[:, 1 : H - 1, H - 1 : H], 0.0)

                zero_border(h1[:], 130)
                zero_border(h2[0][:], 130)
                zero_border(h2[1][:], 130)
                zero_border(h3[:], 66)
                zero_border(h4[:], 34)

                # ---- conv1 ----
                for nt in range(32):
                    ps = cpsum.tile([128, 512], F32, tag="cps")
                    nc.tensor.matmul(ps[:], t_c1[:],
                                     t_xcol[:, nt * 512 : (nt + 1) * 512],
                                     start=True, stop=True)
                    intr = h1[:].rearrange("p (a b) -> p a b", b=130)[
                        :, 1 + nt * 4 : 5 + nt * 4, 1:129]
                    nc.scalar.activation(
                        intr, ps[:].rearrange("p (a b) -> p a b", b=128), ACTF.Copy)
                    nc.vector.bn_stats(t_st6[:, nt * 6 : nt * 6 + 6], ps[:])
                for t, i in [(t_c2, i_c2), (t_c3, i_c3), (t_c4, i_c4),
                             (t_c5, i_c5)]:
                    nc.sync.dma_start(t[:], i[:])
                nc.vector.bn_aggr(t_mv[:, 0:2],
                                  t_st6[:].rearrange("p (g s) -> p g s", s=6))
                bn_allreduce(0, 1, 128)
                h1v = h1[:].rearrange("p (a b) -> p a b", b=130)
                for c4_ in range(4):
                    lrelu_apply(h1v[:, 1 + 32 * c4_ : 33 + 32 * c4_, 1:129],
                                t_ab[:, 0:1], t_ab[:, 1:2])

                # ---- conv2 ----
                if phase_limit < 2:
                    raise _PhaseStop(nc)
                for m in range(2):
                    for nt in range(32):
                        ps = cpsum.tile([128, 512], F32, tag="cps")
                        for off in range(9):
                            ky, kx = off // 3, off % 3
                            rhs = h1v[:, ky + nt * 4 : ky + nt * 4 + 4, kx : kx + 128]
                            nc.tensor.matmul(
                                ps[:],
                                t_c2[:, off * 256 + m * 128 : off * 256 + m * 128 + 128],
                                rhs, start=(off == 0), stop=(off == 8))
                        intr = h2[m][:].rearrange("p (a b) -> p a b", b=130)[
                            :, 1 + nt * 4 : 5 + nt * 4, 1:129]
                        nc.scalar.activation(
                            intr, ps[:].rearrange("p (a b) -> p a b", b=128), ACTF.Copy)
                        nc.vector.bn_stats(t_st6[:, nt * 6 : nt * 6 + 6], ps[:])
                    nc.vector.bn_aggr(t_mv[:, 2 * m : 2 * m + 2],
                                      t_st6[:].rearrange("p (g s) -> p g s", s=6))
                bn_allreduce(1, 2, 128)
                h2v = [h2[m][:].rearrange("p (a b) -> p a b", b=130) for m in range(2)]
                for m in range(2):
                    for c4_ in range(4):
                        lrelu_apply(h2v[m][:, 1 + 32 * c4_ : 33 + 32 * c4_, 1:129],
                                    t_ab[:, 2 * m : 2 * m + 1],
                                    t_ab[:, 2 * m + 1 : 2 * m + 2])

                # ---- conv3 ----
                if phase_limit < 3:
                    raise _PhaseStop(nc)
                for nt in range(8):
                    ps = cpsum.tile([128, 512], F32, tag="cps")
                    first = True
                    for m in range(2):
                        for off in range(16):
                            ky, kx = off // 4, off % 4
                            rhs = h2v[m][:, ky + nt * 16 : ky + nt * 16 + 15 : 2,
                                         kx : kx + 127 : 2]
                            nc.tensor.matmul(
                                ps[:64, :],
                                t_c3[:, (m * 16 + off) * 64 : (m * 16 + off) * 64 + 64],
                                rhs, start=first, stop=(m == 1 and off == 15))
                            first = False
                    intr = h3[:].rearrange("p (a b) -> p a b", b=66)[
                        :, 1 + nt * 8 : 9 + nt * 8, 1:65]
                    nc.scalar.activation(
                        intr, ps[:64, :].rearrange("p (a b) -> p a b", b=64), ACTF.Copy)
                    nc.vector.bn_stats(t_st6[:64, nt * 6 : nt * 6 + 6], ps[:64, :])
                nc.vector.bn_aggr(
                    t_mv[:64, 0:2],
                    t_st6[:64, : 8 * 6].rearrange("p (g s) -> p g s", s=6))
                bn_allreduce(2, 1, 64)
                h3v = h3[:].rearrange("p (a b) -> p a b", b=66)
                lrelu_apply(h3v[:, 1:65, 1:65], t_ab[:64, 0:1], t_ab[:64, 1:2])

                # ---- conv4 ----
                if phase_limit < 4:
                    raise _PhaseStop(nc)
                for nt in range(2):
                    ps = cpsum.tile([128, 512], F32, tag="cps")
                    for off in range(16):
                        ky, kx = off // 4, off % 4
                        rhs = h3v[:, ky + nt * 32 : ky + nt * 32 + 31 : 2, kx : kx + 63 : 2]
                        nc.tensor.matmul(ps[:], t_c4[:, off * 128 : off * 128 + 128],
                                         rhs, start=(off == 0), stop=(off == 15))
                    intr = h4[:].rearrange("p (a b) -> p a b", b=34)[
                        :, 1 + nt * 16 : 17 + nt * 16, 1:33]
                    nc.scalar.activation(
                        intr, ps[:].rearrange("p (a b) -> p a b", b=32), ACTF.Copy)
                    nc.vector.bn_stats(t_st6[:, nt * 6 : nt * 6 + 6], ps[:])
                nc.vector.bn_aggr(
                    t_mv[:, 0:2], t_st6[:, :12].rearrange("p (g s) -> p g s", s=6))
                bn_allreduce(3, 1, 128)
                h4v = h4[:].rearrange("p (a b) -> p a b", b=34)
                lrelu_apply(h4v[:, 1:33, 1:33], t_ab[:, 0:1], t_ab[:, 1:2])

                # ---- conv5 ----
                if phase_limit < 5:
                    raise _PhaseStop(nc)
                for m in range(2):
                    ps = cpsum.tile([128, 512], F32, tag="cps")
                    first = True
                    for off in range(16):
                        ky, kx = off // 4, off % 4
                        rhs = h4v[:, ky : ky + 31 : 2, kx : kx + 31 : 2]
                        nc.tensor.matmul(
                            ps[:, 0:256],
                            t_c5[:, off * 256 + m * 128 : off * 256 + m * 128 + 128],
                            rhs, start=first, stop=(off == 15))
                        first = False
                    nc.scalar.activation(h5[m][:], ps[:, 0:256], ACTF.Copy)
                    nc.vector.bn_stats(t_st6[:, m * 6 : m * 6 + 6], ps[:, 0:256])
                for m in range(2):
                    nc.vector.bn_aggr(
                        t_mv[:, 2 * m : 2 * m + 2],
                        t_st6[:, m * 6 : m * 6 + 6].rearrange("p (g s) -> p g s", s=6))
                bn_allreduce(4, 2, 128)
                for m in range(2):
                    lrelu_apply(h5[m][:], t_ab[:, 2 * m : 2 * m + 1],
                                t_ab[:, 2 * m + 1 : 2 * m + 2])

            if phase_limit < 6:
                raise _PhaseStop(nc)
            # ================= priors (o-major: free = t*903 + o*43 + k) ====
            with tc.tile_pool(name="pri", bufs=1) as pri, \
                 tc.tile_pool(name="route", bufs=1) as rp, \
                 tc.tile_pool(name="scr", bufs=4) as scr:
                P = [[pri.tile([128, 8 * KO], BF16, tag=f"P{g}_{j}", name=f"P{g}_{j}")
                      for j in range(4)] for g in range(2)]
                NG = 4   # tile-groups per cell-group (8 tiles each)
                GT = 8
                L = [[rp.tile([128, GT * 43], F16, tag=f"L{g}_{j}", name=f"L{g}_{j}")
                      for j in range(NG)] for g in range(2)]
                PR = [[rp.tile([128, GT * 43], F16, tag=f"PR{g}_{j}", name=f"PR{g}_{j}")
                       for j in range(NG)] for g in range(2)]
                s_g = [rp.tile([8, KO], F32, tag=f"s_g{g}", name=f"s_g{g}") for g in range(2)]
                sq = [rp.tile([8, KO], F32, tag=f"sq{g}", name=f"sq{g}") for g in range(2)]
                sn = [rp.tile([8, 43], F32, tag=f"sn{g}", name=f"sn{g}") for g in range(2)]
                den = [rp.tile([8, 43], F32, tag=f"den{g}", name=f"den{g}") for g in range(2)]
                phi = [rp.tile([8, 43], F32, tag=f"phi{g}", name=f"phi{g}") for g in range(2)]
                out_f = [rp.tile([8, KO], F32, tag=f"of{g}", name=f"of{g}") for g in range(2)]
                out_bf = [rp.tile([8, KO], BF16, tag=f"ob{g}", name=f"ob{g}") for g in range(2)]
                out_rep = [rp.tile([128, KO], BF16, tag=f"orep{g}", name=f"orep{g}") for g in range(2)]
                for g in range(2):
                    for j in range(NG):
                        nc.vector.memset(L[g][j][:], 0.0)

                sp0 = [None, None]
                with tc.tile_pool(name="ppsum", bufs=1, space="PSUM") as ppsum:
                    cpy = 0
                    for t in range(32):
                        h = t >> 3
                        w = (t >> 1) & 3
                        mblk = t & 1
                        j, tj = t // GT, t % GT
                        rt_t = scr.tile([128, KO], BF16, tag="rt", bufs=4)
                        nc.sync.dma_start(rt_t[:], i_rt[t * 128 : (t + 1) * 128, :])
                        hb = h5[mblk][:].rearrange(
                            "p (hh gy gx ww) -> p hh gy gx ww",
                            hh=4, gy=4, gx=4)
                        for g in range(2):
                            g8 = scr.tile([128, 8], BF16, tag="g8")
                            src = hb[:, h : h + 1, 2 * g : 2 * g + 2, :, w : w + 1]
                            # (p,1,2,4,1) -> (p,2,4)
                            src = src.rearrange("p a b d e -> p (a b) (d e)")
                            nc.gpsimd.tensor_copy(
                                g8[:].rearrange("p (b d) -> p b d", b=2), src)
                            lt = scr.tile([128, 128], BF16, tag="lt")
                            nc.vector.tensor_tensor(
                                lt[:].rearrange("p (n b) -> p n b", b=8),
                                g8[:].rearrange("p (o e) -> p o e", o=1)
                                    .broadcast_to([128, 16, 8]),
                                t_mask[:].rearrange("p (n b) -> p n b", b=8),
                                MULT)
                            pp = ppsum.tile([128, KO], F32, tag="pps", bufs=2)
                            nc.tensor.matmul(pp[:, 0:512], lt[:], rt_t[:, 0:512],
                                             start=True, stop=True,
                                             skip_group_check=True)
                            nc.tensor.matmul(pp[:, 512:KO], lt[:], rt_t[:, 512:KO],
                                             start=True, stop=True,
                                             skip_group_check=True)
                            dst = P[g][j][:, tj * KO : tj * KO + KO]
                            # rotate PSUM->SBUF copies across DVE/Act
                            # (GPSIMD/Pool cannot access PSUM)
                            if cpy % 3 != 1:
                                nc.vector.tensor_copy(dst, pp[:])
                            else:
                                nc.scalar.activation(dst, pp[:], ACTF.Copy)
                            cpy += 1
                            # it0 s-sum: probs are uniform 1/43
                            if t == 0:
                                sp0[g] = ppsum.tile([8, KO], F32, tag=f"sp0_{g}",
                                                    bufs=1, name=f"sp0_{g}")
                            nc.tensor.matmul(sp0[g][:, 0:512], t_selb43[:],
                                             dst[:, 0:512],
                                             start=(t == 0), stop=(t == 31),
                                             skip_group_check=True)
                            nc.tensor.matmul(sp0[g][:, 512:KO], t_selb43[:],
                                             dst[:, 512:KO],
                                             start=(t == 0), stop=(t == 31),
                                             skip_group_check=True)
                    for g in range(2):
                        nc.scalar.activation(s_g[g][:], sp0[g][:], ACTF.Copy)

                # ================= routing =================
                if phase_limit < 7:
                    raise _PhaseStop(nc)

                def squash(g, it, rpsum):
                    """out = s * sqrt(sn)/(1+sn); free dim o-major (o,k)."""
                    nc.scalar.activation(sq[g][:], s_g[g][:], ACTF.Square)
                    nc.vector.tensor_reduce(
                        sn[g][:], sq[g][:].rearrange("p (o k) -> p k o", k=43),
                        AXX, ADD)
                    nc.vector.tensor_scalar_add(den[g][:], sn[g][:], 1.0)
                    nc.vector.reciprocal(den[g][:], den[g][:])
                    nc.scalar.activation(phi[g][:], sn[g][:], ACTF.Sqrt)
                    nc.vector.tensor_tensor(phi[g][:], phi[g][:], den[g][:], MULT)
                    tgt = out_f[g] if it == 2 else out_bf[g]
                    nc.vector.tensor_tensor(
                        tgt[:].rearrange("p (o k) -> p o k", k=43),
                        s_g[g][:].rearrange("p (o k) -> p o k", k=43),
                        phi[g][:].rearrange("p (o k) -> p o k", o=1)
                              .broadcast_to([8, 21, 43]),
                        MULT)  # phi is [8,43]: o=1 split then bcast over o
                    if it == 2:
                        nc.sync.dma_start(o_out[g * 8 : g * 8 + 8, :], tgt[:])
                    else:
                        rpp = rpsum.tile([128, KO], F32, tag="rep", bufs=2)
                        nc.tensor.matmul(
                            rpp[:, 0:512], t_selr[:],
                            out_bf[g][:, 0:512], start=True, stop=True)
                        nc.tensor.matmul(
                            rpp[:, 512:KO], t_selr[:],
                            out_bf[g][:, 512:KO], start=True, stop=True)
                        nc.scalar.activation(out_rep[g][:], rpp[:], ACTF.Copy)

                # (g, j) pairs whose big multiply runs on Pool (balance ~70/30)
                POOL_TM = {(0, 0), (0, 2), (1, 0), (1, 2)}
                POOL_AP = {(1, 3)}

                with tc.tile_pool(name="rpsum", bufs=1, space="PSUM") as rpsum:
                    for g in range(2):
                        squash(g, 0, rpsum)
                    for it in (1, 2):
                        for g in range(2):
                            sp = rpsum.tile([8, KO], F32, tag="sps", bufs=2)
                            for j in range(NG):
                                Pj = P[g][j][:].rearrange(
                                    "p (t o k) -> p t o k", o=21, k=43)
                                # ap = P * out  (f16), bcast out over t
                                ap = scr.tile([128, 8 * KO], F16, tag="ap", bufs=1)
                                ap4 = ap[:].rearrange(
                                    "p (t o k) -> p t o k", o=21, k=43)
                                aeng = nc.gpsimd if (g, j) in POOL_AP else nc.vector
                                with nc.allow_low_precision("logit delta fp16"):
                                    aeng.tensor_tensor(
                                        ap4,
                                        Pj,
                                        out_rep[g][:].rearrange(
                                            "p (a o k) -> p a o k", a=1, k=43)
                                            .broadcast_to([128, 8, 21, 43]),
                                        MULT)
                                    # in-place tree reduce over o (21 = 10+10+1)
                                    for lo, hi, w2 in ((0, 10, 10), (0, 5, 5),
                                                      (0, 2, 2), (0, 1, 1)):
                                        nc.vector.tensor_tensor(
                                            ap4[:, :, lo : lo + w2, :],
                                            ap4[:, :, lo : lo + w2, :],
                                            ap4[:, :, lo + w2 : lo + 2 * w2, :],
                                            ADD)
                                    nc.vector.tensor_tensor(
                                        ap4[:, :, 0:1, :], ap4[:, :, 0:1, :],
                                        ap4[:, :, 4:5, :], ADD)
                                    nc.vector.tensor_tensor(
                                        ap4[:, :, 0:1, :], ap4[:, :, 0:1, :],
                                        ap4[:, :, 20:21, :], ADD)
                                    # L += delta
                                    L4 = L[g][j][:].rearrange(
                                        "p (t a k) -> p t a k", a=1, k=43)
                                    nc.vector.tensor_tensor(
                                        L4, L4, ap4[:, :, 0:1, :], ADD)
                                # softmax over k
                                e8 = scr.tile([128, GT * 43], F16, tag="e8", bufs=1)
                                nc.scalar.activation(e8[:], L[g][j][:], ACTF.Exp)
                                r8 = scr.tile([128, GT], F32, tag="r8")
                                nc.vector.tensor_reduce(
                                    r8[:], e8[:].rearrange("p (t k) -> p t k", k=43),
                                    AXX, ADD)
                                nc.vector.reciprocal(r8[:], r8[:])
                                nc.vector.tensor_tensor(
                                    PR[g][j][:].rearrange("p (t k) -> p t k", k=43),
                                    e8[:].rearrange("p (t k) -> p t k", k=43),
                                    r8[:].rearrange("p (t k) -> p t k", k=1)
                                        .broadcast_to([128, GT, 43]),
                                    MULT)
                                # tm = P * probs (bf16), bcast probs over o
                                tm = scr.tile([128, 8 * KO], BF16, tag="tm", bufs=2)
                                tm4 = tm[:].rearrange(
                                    "p (t o k) -> p t o k", o=21, k=43)
                                teng = nc.gpsimd if (g, j) in POOL_TM else nc.vector
                                teng.tensor_tensor(
                                    tm4,
                                    Pj,
                                    PR[g][j][:].rearrange(
                                        "p (t a k) -> p t a k", a=1, k=43)
                                        .broadcast_to([128, 8, 21, 43]),
                                    MULT)
                                for tj in range(GT):
                                    rhs_t = tm[:, tj * KO : tj * KO + KO]
                                    nc.tensor.matmul(
                                        sp[:, 0:512], t_selb[:], rhs_t[:, 0:512],
                                        start=(j == 0 and tj == 0),
                                        stop=(j == NG - 1 and tj == GT - 1))
                                    nc.tensor.matmul(
                                        sp[:, 512:KO], t_selb[:], rhs_t[:, 512:KO],
                                        start=(j == 0 and tj == 0),
                                        stop=(j == NG - 1 and tj == GT - 1))
                            nc.scalar.activation(s_g[g][:], sp[:], ACTF.Copy)
                            squash(g, it, rpsum)
    _spill_extra_waits(nc)
    return nc


_CACHED = {}


def _get_bass():
    if "nc" not in _CACHED:
        _CACHED["nc"] = _build_bass()
    return _CACHED["nc"]


def kernel(**inputs):
    from concourse.bass_utils import run_bass_kernel_spmd

    d = {k: np.asarray(v) for k, v in inputs.items()}
    shared = _prep_shared(d)
    x = np.asarray(d["x"], np.float32)

    nc = _get_bass()
    in_maps = []
    for c in range(NCORES):
        m = dict(shared)
        m["xcol"] = _bf(_im2col(x[c]))
        in_maps.append(m)

    import os
    trace = bool(os.environ.get("DCAPS_TRACE"))
    res = run_bass_kernel_spmd(
        nc, in_maps, core_ids=list(range(NCORES)), trace=trace)
    _CACHED["last_results"] = res
    _CACHED["last_in_maps"] = in_maps

    out = np.empty((NCORES, 4, 4, N_CLASSES, 21), np.float32)
    for c in range(NCORES):
        r = np.asarray(res.results[c]["out"])  # (16, 903) o-major: col = o*43+k
        for gy in range(4):
            for gx in range(4):
                cell = (gy >> 1) * 8 + (gy & 1) * 4 + gx
                out[c, gy, gx] = r[cell].reshape(21, N_CLASSES).T
    return out



# revision 20
# speedup vs baseline: 1.2401x; 1.0998x over previous
"""DarkCapsuleNet on 8 Trainium2 NeuronCores.

Data-parallel over batch (B=8, one image per core). The conv+BN+LReLU
backbone runs per core on its image; BN batch statistics are combined
across cores with tiny AllReduces (per-channel [mean, E[x^2]] sums). The
capsule-routing stage is independent per (grid-cell, image), so each core
routes its own 16 cells entirely in SBUF.

Convs are direct convolutions: matmuls accumulated over kernel offsets with
input channels on the contraction dim, bf16 operands, fp32 PSUM. Priors use
a block-diagonal lhsT built on-chip with one masked DVE multiply per tile,
so the 8-wide capsule contraction still runs as full 128-wide matmuls.
"""

import numpy as np
import ml_dtypes


class _PhaseStop(Exception):
    def __init__(self, nc):
        self.nc = nc

N_CLASSES = 43
KO = N_CLASSES * 21  # 903
EPS = 1e-5
NCORES = 8

_BF16 = ml_dtypes.bfloat16


# ---------------------------------------------------------------------------
# Workaround: this walrus build accepts at most ONE sem wait on a TPB_CTRL
# Drain instruction; Tile's epilogue drain carries one wait per HW-DMA queue.
# Split the extra waits onto standalone SP nops (same engine, before the
# all-engine barrier, so semantics are unchanged).
# ---------------------------------------------------------------------------
def _install_tile_drain_fix():
    import concourse.tile as tile_mod
    import concourse.mybir as mybir
    from concourse.vector_clock import ScopedClock

    if getattr(tile_mod.TileContext, "_drain_fix_installed", False):
        return

    def _patched(self, tick_clock, wait_clock):
        drain_inst = self.nc.sync.drain()
        wait_clock.add_sem_waits(
            drain_inst.ins, ScopedClock({None: tick_clock.global_clock})
        )
        raw = drain_inst.ins
        si = getattr(raw, "sync_info", None)
        if si is not None and si.on_wait is not None and len(si.on_wait) > 1:
            waits = list(si.on_wait)
            si.on_wait = waits[-1:]
            for w in waits[:-1]:
                nop = self.nc.sync.nop(nofuse=True, hint="split_drain_wait")
                nsi = getattr(nop.ins, "sync_info", None)
                if nsi is None:
                    nop.ins.sync_info = mybir.SyncInfo(on_update=[], on_wait=[w])
                else:
                    nw = list(nsi.on_wait) if nsi.on_wait else []
                    nw.append(w)
                    nsi.on_wait = nw
        self.nc.all_engine_barrier()
        assert self.sems is not None
        popped = self.nc._tile_sem_poison_stack.pop()
        assert popped is self._sem_poison
        self.nc.clear_and_free_semaphores(list(self.sems.allocated().values()))
        self.nc.all_engine_barrier()

    tile_mod.TileContext._drain_and_barrier = _patched
    tile_mod.TileContext._drain_fix_installed = True


# ---------------------------------------------------------------------------
# Host-side layout prep
# ---------------------------------------------------------------------------
def _bf(x):
    return np.ascontiguousarray(np.asarray(x, np.float32).astype(_BF16))


def _im2col(img):
    # img (3,128,128) f32 -> (27,16384), rows (ci,ky,kx)
    xp = np.zeros((3, 130, 130), np.float32)
    xp[:, 1:129, 1:129] = img
    cols = np.empty((3, 3, 3, 128, 128), np.float32)
    for ky in range(3):
        for kx in range(3):
            cols[:, ky, kx] = xp[:, ky : ky + 128, kx : kx + 128]
    return cols.reshape(27, 16384)


def _prep_shared(d):
    c1h = np.asarray(d["c1w"], np.float32).reshape(128, 27).T.copy()
    c2h = np.asarray(d["c2w"], np.float32).transpose(2, 3, 1, 0).reshape(9, 128, 256)
    c2h = np.concatenate(list(c2h), axis=1)  # (128, 9*256)
    c3t = np.asarray(d["c3w"], np.float32).transpose(1, 2, 3, 0)  # (256,4,4,64)
    c3h = np.concatenate(
        [c3t[m * 128 : (m + 1) * 128].reshape(128, 16 * 64) for m in range(2)], axis=1
    )  # (128, 2048)
    c4h = np.asarray(d["c4w"], np.float32).transpose(1, 2, 3, 0).reshape(64, 16 * 128)
    c5h = np.asarray(d["c5w"], np.float32).transpose(1, 2, 3, 0).reshape(128, 16 * 256)

    rw = np.asarray(d["rw"], np.float32)  # (512,43,8,21)
    # o-major columns: col = o*43 + k (so k is innermost => packed 2-byte
    # innermost dims everywhere in routing => DVE 2x perf mode)
    rt = rw.transpose(0, 2, 3, 1).reshape(512 * 8, KO)  # row = n*8+i, col = o*43+k

    gb = np.zeros((128, 14), np.float32)
    gb[:, 0] = d["g1"]; gb[:, 1] = d["b1"]
    gb[:, 2] = d["g2"][:128]; gb[:, 3] = d["b2"][:128]
    gb[:, 4] = d["g2"][128:]; gb[:, 5] = d["b2"][128:]
    gb[:64, 6] = d["g3"]; gb[:64, 7] = d["b3"]
    gb[:, 8] = d["g4"]; gb[:, 9] = d["b4"]
    gb[:, 10] = d["g5"][:128]; gb[:, 11] = d["b5"][:128]
    gb[:, 12] = d["g5"][128:]; gb[:, 13] = d["b5"][128:]

    mask = np.zeros((128, 128), np.float32)
    for p in range(128):
        mask[p, (p >> 3) * 8 : (p >> 3) * 8 + 8] = 1.0
    selb = np.zeros((128, 8), np.float32)
    for p in range(128):
        selb[p, p & 7] = 1.0
    selr = np.zeros((8, 128), np.float32)  # [b, ns*8 + b]
    for ns in range(16):
        for b in range(8):
            selr[b, ns * 8 + b] = 1.0
    def _f16(x):
        return np.ascontiguousarray(np.asarray(x, np.float16))
    return dict(
        c1wT=_bf(c1h), c2wT=_bf(c2h), c3wT=_bf(c3h), c4wT=_bf(c4h), c5wT=_bf(c5h),
        RT=_f16(rt), gb=gb, MASK=_f16(mask), SELB=_f16(selb),
        SELB43=_f16(selb / 43.0), SELR=_f16(selr),
    )


# ---------------------------------------------------------------------------
# Bass program (identical on every core)
# ---------------------------------------------------------------------------
def _spill_extra_waits(nc):
    """This walrus codegen accepts at most one semaphore wait per TPB
    instruction. Tile can attach several. Move the extras onto fresh NoOp
    instructions inserted just before the owner on the same engine."""
    import concourse.mybir as mybir

    uid = [0]
    for f in nc.m.functions:
        for bb in f.blocks:
            il = bb.instructions
            out = []
            changed = False
            for inst in il:
                si = getattr(inst, "sync_info", None)
                waits = list(si.on_wait) if si is not None and si.on_wait else []
                if len(waits) > 1:
                    for w in waits[:-1]:
                        uid[0] += 1
                        nop = mybir.InstNoOp(name=f"waitspill-{uid[0]}", ins=[], outs=[])
                        nop.engine = inst.engine
                        nop.sync_info = mybir.SyncInfo(on_update=[], on_wait=[w])
                        out.append(nop)
                    si.on_wait = waits[-1:]
                    changed = True
                out.append(inst)
            if changed:
                bb.instructions = out


def _build_bass(phase_limit=99):
    import concourse.bass as bass
    import concourse.mybir as mybir
    from concourse import tile

    _install_tile_drain_fix()

    F32 = mybir.dt.float32
    BF16 = mybir.dt.bfloat16
    F16 = mybir.dt.float16
    ADD = mybir.AluOpType.add
    MULT = mybir.AluOpType.mult
    SUB = mybir.AluOpType.subtract
    ACTF = mybir.ActivationFunctionType
    AXX = mybir.AxisListType.X

    nc = bass.Bass(num_devices=NCORES)
    dp = nc.declare_dram_parameter
    i_xcol = dp("xcol", [27, 16384], BF16, isOutput=False)
    i_c1 = dp("c1wT", [27, 128], BF16, isOutput=False)
    i_c2 = dp("c2wT", [128, 2304], BF16, isOutput=False)
    i_c3 = dp("c3wT", [128, 2048], BF16, isOutput=False)
    i_c4 = dp("c4wT", [64, 2048], BF16, isOutput=False)
    i_c5 = dp("c5wT", [128, 4096], BF16, isOutput=False)
    i_rt = dp("RT", [4096, KO], F16, isOutput=False)
    i_gb = dp("gb", [128, 14], F32, isOutput=False)
    i_mask = dp("MASK", [128, 128], F16, isOutput=False)
    i_selb = dp("SELB", [128, 8], F16, isOutput=False)
    i_selb43 = dp("SELB43", [128, 8], F16, isOutput=False)
    i_selr = dp("SELR", [8, 128], F16, isOutput=False)
    o_out = dp("out", [16, KO], F32, isOutput=True)


    with tile.TileContext(nc) as tc:
        with tc.tile_pool(name="const", bufs=1) as const, \
             tc.tile_pool(name="dram", bufs=1, space="DRAM") as dram:
            t_gb = const.tile([128, 14], F32)
            t_mask = const.tile([128, 128], F16)
            t_selb = const.tile([128, 8], F16)
            t_selb43 = const.tile([128, 8], F16)
            t_selr = const.tile([8, 128], F16)
            h5 = [const.tile([128, 256], BF16, tag=f"h5_{m}", name=f"h5_{m}") for m in range(2)]
            t_st6 = const.tile([128, 32 * 6], F32)
            t_mv = const.tile([128, 4], F32)
            t_ab = const.tile([128, 4], F32)
            t_sc = const.tile([128, 2], F32)
            for t, i in [(t_gb, i_gb), (t_mask, i_mask), (t_selb, i_selb),
                         (t_selb43, i_selb43), (t_selr, i_selr)]:
                nc.sync.dma_start(t[:], i[:])

            ar_in = [dram.tile([128, 4], F32, tag=f"ari{i}", name=f"ari{i}") for i in range(5)]
            ar_out = [dram.tile([8, 512], F32, tag=f"aro{i}", name=f"aro{i}") for i in range(5)]
            t_ag = const.tile([128, 32], F32)

            def bn_allreduce(layer, nch_tiles, npart):
                """t_mv holds per-core [m0,v0,m1,v1]; leaves [a0,b0,a1,b1] in t_ab."""
                for mt in range(nch_tiles):
                    m = t_mv[:npart, 2 * mt : 2 * mt + 1]
                    v = t_mv[:npart, 2 * mt + 1 : 2 * mt + 2]
                    s1 = t_sc[:npart, 0:1]
                    nc.scalar.activation(s1, m, ACTF.Square)
                    nc.vector.tensor_tensor(v, v, s1, ADD)  # v := E[x^2] local
                nc.sync.dma_start(ar_in[layer][:], t_mv[:])
                nc.gpsimd.collective_compute(
                    "AllGather", mybir.AluOpType.bypass,
                    ins=[ar_in[layer][:]], outs=[ar_out[layer][:]],
                    replica_groups=[list(range(NCORES))],
                )
                nc.sync.dma_start(
                    t_ag[:].rearrange("p (g c) -> p g c", c=4),
                    ar_out[layer][:].rearrange("g (p c) -> p g c", c=4),
                )
                nc.vector.tensor_reduce(
                    t_mv[:, 0:4], t_ag[:].rearrange("p (g c) -> p c g", c=4),
                    AXX, ADD)
                for mt in range(nch_tiles):
                    m = t_mv[:npart, 2 * mt : 2 * mt + 1]
                    q = t_mv[:npart, 2 * mt + 1 : 2 * mt + 2]
                    a = t_ab[:npart, 2 * mt : 2 * mt + 1]
                    b = t_ab[:npart, 2 * mt + 1 : 2 * mt + 2]
                    s1 = t_sc[:npart, 0:1]
                    nc.vector.tensor_scalar_mul(m, m, 1.0 / NCORES)
                    nc.vector.tensor_scalar_mul(q, q, 1.0 / NCORES)
                    nc.scalar.activation(s1, m, ACTF.Square)
                    nc.vector.tensor_tensor(q, q, s1, SUB)       # gvar
                    nc.vector.tensor_scalar_add(q, q, EPS)
                    nc.vector.reciprocal(s1, q)
                    nc.scalar.activation(s1, s1, ACTF.Sqrt)      # rsqrt(var+eps)
                    gcol = (0, 2, 6, 8, 10)[layer] + 2 * mt
                    nc.vector.tensor_tensor(a, t_gb[:npart, gcol : gcol + 1], s1, MULT)
                    nc.vector.tensor_tensor(s1, a, m, MULT)
                    nc.vector.tensor_tensor(b, t_gb[:npart, gcol + 1 : gcol + 2], s1, SUB)

            def lrelu_apply(view, scale, bias):
                nc.scalar.activation(view, view, ACTF.Prelu,
                                     bias=bias, scale=scale, alpha=0.1)

            # ================= conv backbone =================
            with tc.tile_pool(name="wpool", bufs=1) as wp, \
                 tc.tile_pool(name="xpool", bufs=1) as xp, \
                 tc.tile_pool(name="acts", bufs=1) as acts, \
                 tc.tile_pool(name="cpsum", bufs=4, space="PSUM") as cpsum:
                t_c2 = wp.tile([128, 2304], BF16)
                t_c3 = wp.tile([128, 2048], BF16)
                t_c4 = wp.tile([64, 2048], BF16)
                t_c5 = wp.tile([128, 4096], BF16)
                t_c1 = xp.tile([27, 128], BF16)
                t_xcol = xp.tile([27, 16384], BF16)
                nc.sync.dma_start(t_c1[:], i_c1[:])
                for ch in range(4):
                    nc.sync.dma_start(t_xcol[:, ch * 4096 : (ch + 1) * 4096],
                                      i_xcol[:, ch * 4096 : (ch + 1) * 4096])

                h1 = acts.tile([128, 130 * 130], BF16)
                h2 = [acts.tile([128, 130 * 130], BF16, tag=f"h2_{m}", name=f"h2_{m}") for m in range(2)]
                h3 = acts.tile([64, 66 * 66], BF16)
                h4 = acts.tile([128, 34 * 34], BF16)

                def zero_border(tile_ap, H):
                    v = tile_ap.rearrange("p (a b) -> p a b", b=H)
                    nc.gpsimd.memset(v[:, 0:1, :], 0.0)
                    nc.gpsimd.memset(v[:, H - 1 : H, :], 0.0)
                    nc.gpsimd.memset(v[:, 1 : H - 1, 0:1], 0.0)
                    nc.gpsimd.memset(v[:, 1 : H - 1, H - 1 : H], 0.0)

                zero_border(h1[:], 130)
                zero_border(h2[0][:], 130)
                zero_border(h2[1][:], 130)
                zero_border(h3[:], 66)
                zero_border(h4[:], 34)

                # ---- conv1 ----
                for nt in range(32):
                    ps = cpsum.tile([128, 512], F32, tag="cps")
                    nc.tensor.matmul(ps[:], t_c1[:],
                                     t_xcol[:, nt * 512 : (nt + 1) * 512],
                                     start=True, stop=True)
                    intr = h1[:].rearrange("p (a b) -> p a b", b=130)[
                        :, 1 + nt * 4 : 5 + nt * 4, 1:129]
                    nc.scalar.activation(
                        intr, ps[:].rearrange("p (a b) -> p a b", b=128), ACTF.Copy)
                    nc.vector.bn_stats(t_st6[:, nt * 6 : nt * 6 + 6], ps[:])
                for t, i in [(t_c2, i_c2), (t_c3, i_c3), (t_c4, i_c4),
                             (t_c5, i_c5)]:
                    nc.sync.dma_start(t[:], i[:])
                nc.vector.bn_aggr(t_mv[:, 0:2],
                                  t_st6[:].rearrange("p (g s) -> p g s", s=6))
                bn_allreduce(0, 1, 128)
                h1v = h1[:].rearrange("p (a b) -> p a b", b=130)
                for c4_ in range(4):
                    lrelu_apply(h1v[:, 1 + 32 * c4_ : 33 + 32 * c4_, 1:129],
                                t_ab[:, 0:1], t_ab[:, 1:2])

                # ---- conv2 ----
                if phase_limit < 2:
                    raise _PhaseStop(nc)
                for m in range(2):
                    for nt in range(32):
                        ps = cpsum.tile([128, 512], F32, tag="cps")
                        for off in range(9):
                            ky, kx = off // 3, off % 3
                            rhs = h1v[:, ky + nt * 4 : ky + nt * 4 + 4, kx : kx + 128]
                            nc.tensor.matmul(
                                ps[:],
                                t_c2[:, off * 256 + m * 128 : off * 256 + m * 128 + 128],
                                rhs, start=(off == 0), stop=(off == 8))
                        intr = h2[m][:].rearrange("p (a b) -> p a b", b=130)[
                            :, 1 + nt * 4 : 5 + nt * 4, 1:129]
                        nc.scalar.activation(
                            intr, ps[:].rearrange("p (a b) -> p a b", b=128), ACTF.Copy)
                        nc.vector.bn_stats(t_st6[:, nt * 6 : nt * 6 + 6], ps[:])
                    nc.vector.bn_aggr(t_mv[:, 2 * m : 2 * m + 2],
                                      t_st6[:].rearrange("p (g s) -> p g s", s=6))
                bn_allreduce(1, 2, 128)
                h2v = [h2[m][:].rearrange("p (a b) -> p a b", b=130) for m in range(2)]
                for m in range(2):
                    for c4_ in range(4):
                        lrelu_apply(h2v[m][:, 1 + 32 * c4_ : 33 + 32 * c4_, 1:129],
                                    t_ab[:, 2 * m : 2 * m + 1],
                                    t_ab[:, 2 * m + 1 : 2 * m + 2])

                # ---- conv3 ----
                if phase_limit < 3:
                    raise _PhaseStop(nc)
                for nt in range(8):
                    ps = cpsum.tile([128, 512], F32, tag="cps")
                    first = True
                    for m in range(2):
                        for off in range(16):
                            ky, kx = off // 4, off % 4
                            rhs = h2v[m][:, ky + nt * 16 : ky + nt * 16 + 15 : 2,
                                         kx : kx + 127 : 2]
                            nc.tensor.matmul(
                                ps[:64, :],
                                t_c3[:, (m * 16 + off) * 64 : (m * 16 + off) * 64 + 64],
                                rhs, start=first, stop=(m == 1 and off == 15))
                            first = False
                    intr = h3[:].rearrange("p (a b) -> p a b", b=66)[
                        :, 1 + nt * 8 : 9 + nt * 8, 1:65]
                    nc.scalar.activation(
                        intr, ps[:64, :].rearrange("p (a b) -> p a b", b=64), ACTF.Copy)
                    nc.vector.bn_stats(t_st6[:64, nt * 6 : nt * 6 + 6], ps[:64, :])
                nc.vector.bn_aggr(
                    t_mv[:64, 0:2],
                    t_st6[:64, : 8 * 6].rearrange("p (g s) -> p g s", s=6))
                bn_allreduce(2, 1, 64)
                h3v = h3[:].rearrange("p (a b) -> p a b", b=66)
                lrelu_apply(h3v[:, 1:65, 1:65], t_ab[:64, 0:1], t_ab[:64, 1:2])

                # ---- conv4 ----
                if phase_limit < 4:
                    raise _PhaseStop(nc)
                for nt in range(2):
                    ps = cpsum.tile([128, 512], F32, tag="cps")
                    for off in range(16):
                        ky, kx = off // 4, off % 4
                        rhs = h3v[:, ky + nt * 32 : ky + nt * 32 + 31 : 2, kx : kx + 63 : 2]
                        nc.tensor.matmul(ps[:], t_c4[:, off * 128 : off * 128 + 128],
                                         rhs, start=(off == 0), stop=(off == 15))
                    intr = h4[:].rearrange("p (a b) -> p a b", b=34)[
                        :, 1 + nt * 16 : 17 + nt * 16, 1:33]
                    nc.scalar.activation(
                        intr, ps[:].rearrange("p (a b) -> p a b", b=32), ACTF.Copy)
                    nc.vector.bn_stats(t_st6[:, nt * 6 : nt * 6 + 6], ps[:])
                nc.vector.bn_aggr(
                    t_mv[:, 0:2], t_st6[:, :12].rearrange("p (g s) -> p g s", s=6))
                bn_allreduce(3, 1, 128)
                h4v = h4[:].rearrange("p (a b) -> p a b", b=34)
                lrelu_apply(h4v[:, 1:33, 1:33], t_ab[:, 0:1], t_ab[:, 1:2])

                # ---- conv5 ----
                if phase_limit < 5:
                    raise _PhaseStop(nc)
                for m in range(2):
                    ps = cpsum.tile([128, 512], F32, tag="cps")
                    first = True
                    for off in range(16):
                        ky, kx = off // 4, off % 4
                        rhs = h4v[:, ky : ky + 31 : 2, kx : kx + 31 : 2]
                        nc.tensor.matmul(
                            ps[:, 0:256],
                            t_c5[:, off * 256 + m * 128 : off * 256 + m * 128 + 128],
                            rhs, start=first, stop=(off == 15))
                        first = False
                    nc.scalar.activation(h5[m][:], ps[:, 0:256], ACTF.Copy)
                    nc.vector.bn_stats(t_st6[:, m * 6 : m * 6 + 6], ps[:, 0:256])
                for m in range(2):
                    nc.vector.bn_aggr(
                        t_mv[:, 2 * m : 2 * m + 2],
                        t_st6[:, m * 6 : m * 6 + 6].rearrange("p (g s) -> p g s", s=6))
                bn_allreduce(4, 2, 128)
                for m in range(2):
                    lrelu_apply(h5[m][:], t_ab[:, 2 * m : 2 * m + 1],
                                t_ab[:, 2 * m + 1 : 2 * m + 2])

            if phase_limit < 6:
                raise _PhaseStop(nc)
            # ================= priors (o-major: free = t*903 + o*43 + k) ====
            with tc.tile_pool(name="pri", bufs=1) as pri, \
                 tc.tile_pool(name="route", bufs=1) as rp, \
                 tc.tile_pool(name="scr", bufs=4) as scr:
                P = [[pri.tile([128, 8 * KO], F16, tag=f"P{g}_{j}", name=f"P{g}_{j}")
                      for j in range(4)] for g in range(2)]
                NG = 4   # tile-groups per cell-group (8 tiles each)
                GT = 8
                L = [[rp.tile([128, GT * 43], F16, tag=f"L{g}_{j}", name=f"L{g}_{j}")
                      for j in range(NG)] for g in range(2)]
                s_g = [rp.tile([8, KO], F32, tag=f"s_g{g}", name=f"s_g{g}") for g in range(2)]
                sn = [rp.tile([8, 43], F32, tag=f"sn{g}", name=f"sn{g}") for g in range(2)]
                den = [rp.tile([8, 43], F32, tag=f"den{g}", name=f"den{g}") for g in range(2)]
                phi = [rp.tile([8, 43], F32, tag=f"phi{g}", name=f"phi{g}") for g in range(2)]
                out_f = [rp.tile([8, KO], F32, tag=f"of{g}", name=f"of{g}") for g in range(2)]
                out_bf = [rp.tile([8, KO], F16, tag=f"ob{g}", name=f"ob{g}") for g in range(2)]
                out_rep = [rp.tile([128, KO], BF16, tag=f"orep{g}", name=f"orep{g}") for g in range(2)]
                for g in range(2):
                    for j in range(NG):
                        nc.vector.memset(L[g][j][:], 0.0)

                sp0 = [None, None]
                with tc.tile_pool(name="ppsum", bufs=1, space="PSUM") as ppsum:
                    cpy = 0
                    for t in range(32):
                        h = t >> 3
                        w = (t >> 1) & 3
                        mblk = t & 1
                        j, tj = t // GT, t % GT
                        rt_t = scr.tile([128, KO], F16, tag="rt", bufs=3)
                        nc.sync.dma_start(rt_t[:], i_rt[t * 128 : (t + 1) * 128, :])
                        hb = h5[mblk][:].rearrange(
                            "p (hh gy gx ww) -> p hh gy gx ww",
                            hh=4, gy=4, gx=4)
                        for g in range(2):
                            g8 = scr.tile([128, 8], F16, tag="g8")
                            src = hb[:, h : h + 1, 2 * g : 2 * g + 2, :, w : w + 1]
                            # (p,1,2,4,1) -> (p,2,4)
                            src = src.rearrange("p a b d e -> p (a b) (d e)")
                            nc.gpsimd.tensor_copy(
                                g8[:].rearrange("p (b d) -> p b d", b=2), src)
                            lt = scr.tile([128, 128], F16, tag="lt", bufs=2)
                            nc.vector.tensor_tensor(
                                lt[:].rearrange("p (n b) -> p n b", b=8),
                                g8[:].rearrange("p (o e) -> p o e", o=1)
                                    .broadcast_to([128, 16, 8]),
                                t_mask[:].rearrange("p (n b) -> p n b", b=8),
                                MULT)
                            pp = ppsum.tile([128, KO], F32, tag="pps", bufs=2)
                            nc.tensor.matmul(pp[:, 0:512], lt[:], rt_t[:, 0:512],
                                             start=True, stop=True,
                                             skip_group_check=True)
                            nc.tensor.matmul(pp[:, 512:KO], lt[:], rt_t[:, 512:KO],
                                             start=True, stop=True,
                                             skip_group_check=True)
                            dst = P[g][j][:, tj * KO : tj * KO + KO]
                            # rotate PSUM->SBUF copies across Act/DVE
                            # (GPSIMD/Pool cannot access PSUM; Act is cheaper
                            # than DVE here since f32 psum reads get no 2x)
                            if cpy % 3 == 1:
                                nc.vector.tensor_copy(dst, pp[:])
                            else:
                                nc.scalar.activation(dst, pp[:], ACTF.Copy)
                            cpy += 1
                            # it0 s-sum: probs are uniform 1/43
                            if t == 0:
                                sp0[g] = ppsum.tile([8, KO], F32, tag=f"sp0_{g}",
                                                    bufs=1, name=f"sp0_{g}")
                            nc.tensor.matmul(sp0[g][:, 0:512], t_selb43[:],
                                             dst[:, 0:512],
                                             start=(t == 0), stop=(t == 31),
                                             skip_group_check=True)
                            nc.tensor.matmul(sp0[g][:, 512:KO], t_selb43[:],
                                             dst[:, 512:KO],
                                             start=(t == 0), stop=(t == 31),
                                             skip_group_check=True)
                    for g in range(2):
                        nc.scalar.activation(s_g[g][:], sp0[g][:], ACTF.Copy)

                # ================= routing =================
                if phase_limit < 7:
                    raise _PhaseStop(nc)

                def squash(g, it, rpsum):
                    """out = s * sqrt(sn)/(1+sn); free dim o-major (o,k)."""
                    nc.scalar.activation(out_f[g][:], s_g[g][:], ACTF.Square)
                    nc.vector.tensor_reduce(
                        sn[g][:], out_f[g][:].rearrange("p (o k) -> p k o", k=43),
                        AXX, ADD)
                    nc.vector.tensor_scalar_add(den[g][:], sn[g][:], 1.0)
                    nc.vector.reciprocal(den[g][:], den[g][:])
                    nc.scalar.activation(phi[g][:], sn[g][:], ACTF.Sqrt)
                    nc.vector.tensor_tensor(phi[g][:], phi[g][:], den[g][:], MULT)
                    tgt = out_f[g] if it == 2 else out_bf[g]
                    nc.vector.tensor_tensor(
                        tgt[:].rearrange("p (o k) -> p o k", k=43),
                        s_g[g][:].rearrange("p (o k) -> p o k", k=43),
                        phi[g][:].rearrange("p (o k) -> p o k", o=1)
                              .broadcast_to([8, 21, 43]),
                        MULT)  # phi is [8,43]: o=1 split then bcast over o
                    if it == 2:
                        nc.sync.dma_start(o_out[g * 8 : g * 8 + 8, :], tgt[:])
                    else:
                        rpp = rpsum.tile([128, KO], F32, tag="rep", bufs=2)
                        nc.tensor.matmul(
                            rpp[:, 0:512], t_selr[:],
                            out_bf[g][:, 0:512], start=True, stop=True)
                        nc.tensor.matmul(
                            rpp[:, 512:KO], t_selr[:],
                            out_bf[g][:, 512:KO], start=True, stop=True)
                        nc.scalar.activation(out_rep[g][:], rpp[:], ACTF.Copy)

                # Pool (gpsimd) takes a fixed slice of the t-tiles of every big
                # multiply; DVE (2x perf mode, ~3.8x faster per elem) takes the
                # rest plus the tree reduce. Fine slices => tight overlap.
                AP_POOL_T = 2   # pool gets t-tiles [6:8] of each ap
                TM_POOL_T = 3   # pool gets t-tiles [5:8] of each tm

                with tc.tile_pool(name="rpsum", bufs=1, space="PSUM") as rpsum:
                    for g in range(2):
                        squash(g, 0, rpsum)
                    def emit_ap(g, j):
                        """ap = P * out (f16), bcast out over t; pool takes the
                        last AP_POOL_T t-tiles, DVE the rest."""
                        Pj = P[g][j][:].rearrange(
                            "p (t o k) -> p t o k", o=21, k=43)
                        ap = scr.tile([128, 8 * KO], F16, tag="big", bufs=4,
                                      name=f"ap{g}{j}")
                        ap4 = ap[:].rearrange("p (t o k) -> p t o k", o=21, k=43)
                        orr = out_rep[g][:].rearrange(
                            "p (a o k) -> p a o k", a=1, k=43)
                        ts = 8 - AP_POOL_T
                        with nc.allow_low_precision("logit delta fp16"):
                            nc.gpsimd.tensor_tensor(
                                ap4[:, ts:8], Pj[:, ts:8],
                                orr.broadcast_to([128, AP_POOL_T, 21, 43]), MULT)
                            nc.vector.tensor_tensor(
                                ap4[:, 0:ts], Pj[:, 0:ts],
                                orr.broadcast_to([128, ts, 21, 43]), MULT)
                        return ap

                    for it in (1, 2):
                        for g in range(2):
                            sp = rpsum.tile([8, KO], F32, tag="sps", bufs=2)
                            aps = [emit_ap(g, 0), emit_ap(g, 1)]
                            for j in range(NG):
                                Pj = P[g][j][:].rearrange(
                                    "p (t o k) -> p t o k", o=21, k=43)
                                ap4 = aps[j][:].rearrange(
                                    "p (t o k) -> p t o k", o=21, k=43)
                                with nc.allow_low_precision("logit delta fp16"):
                                    # in-place tree reduce over o (21 = 10+10+1)
                                    for lo, w2 in ((0, 10), (0, 5),
                                                   (0, 2), (0, 1)):
                                        nc.vector.tensor_tensor(
                                            ap4[:, :, lo : lo + w2, :],
                                            ap4[:, :, lo : lo + w2, :],
                                            ap4[:, :, lo + w2 : lo + 2 * w2, :],
                                            ADD)
                                    nc.vector.tensor_tensor(
                                        ap4[:, :, 0:1, :], ap4[:, :, 0:1, :],
                                        ap4[:, :, 4:5, :], ADD)
                                    nc.vector.tensor_tensor(
                                        ap4[:, :, 0:1, :], ap4[:, :, 0:1, :],
                                        ap4[:, :, 20:21, :], ADD)
                                    # L += delta
                                    L4 = L[g][j][:].rearrange(
                                        "p (t a k) -> p t a k", a=1, k=43)
                                    nc.vector.tensor_tensor(
                                        L4, L4, ap4[:, :, 0:1, :], ADD)
                                # softmax over k
                                e8 = scr.tile([128, GT * 43], F16, tag="e8", bufs=2)
                                nc.scalar.activation(e8[:], L[g][j][:], ACTF.Exp)
                                r8 = scr.tile([128, GT], F32, tag="r8", bufs=2)
                                nc.vector.tensor_reduce(
                                    r8[:], e8[:].rearrange("p (t k) -> p t k", k=43),
                                    AXX, ADD)
                                nc.vector.reciprocal(r8[:], r8[:])
                                nc.vector.tensor_tensor(
                                    e8[:].rearrange("p (t k) -> p t k", k=43),
                                    e8[:].rearrange("p (t k) -> p t k", k=43),
                                    r8[:].rearrange("p (t k) -> p t k", k=1)
                                        .broadcast_to([128, GT, 43]),
                                    MULT)
                                # prefetch ap for j+2 before tm(j) so the
                                # pool never blocks the next tree
                                if j + 2 < NG:
                                    aps.append(emit_ap(g, j + 2))
                                # tm = P * probs (f16), bcast probs over o
                                tm = scr.tile([128, 8 * KO], F16, tag="big",
                                              bufs=4, name=f"tm{g}{j}")
                                tm4 = tm[:].rearrange(
                                    "p (t o k) -> p t o k", o=21, k=43)
                                prr = e8[:].rearrange(
                                    "p (t a k) -> p t a k", a=1, k=43)
                                ts = 8 - TM_POOL_T
                                nc.gpsimd.tensor_tensor(
                                    tm4[:, ts:8], Pj[:, ts:8],
                                    prr[:, ts:8].broadcast_to(
                                        [128, TM_POOL_T, 21, 43]),
                                    MULT)
                                nc.vector.tensor_tensor(
                                    tm4[:, 0:ts], Pj[:, 0:ts],
                                    prr[:, 0:ts].broadcast_to([128, ts, 21, 43]),
                                    MULT)
                                for tj in range(GT):
                                    rhs_t = tm[:, tj * KO : tj * KO + KO]
                                    nc.tensor.matmul(
                                        sp[:, 0:512], t_selb[:], rhs_t[:, 0:512],
                                        start=(j == 0 and tj == 0),
                                        stop=(j == NG - 1 and tj == GT - 1))
                                    nc.tensor.matmul(
                                        sp[:, 512:KO], t_selb[:], rhs_t[:, 512:KO],
                                        start=(j == 0 and tj == 0),
                                        stop=(j == NG - 1 and tj == GT - 1))
                            nc.scalar.activation(s_g[g][:], sp[:], ACTF.Copy)
                            squash(g, it, rpsum)
    _spill_extra_waits(nc)
    return nc


_CACHED = {}


def _get_bass():
    if "nc" not in _CACHED:
        _CACHED["nc"] = _build_bass()
    return _CACHED["nc"]


def kernel(**inputs):
    from concourse.bass_utils import run_bass_kernel_spmd

    d = {k: np.asarray(v) for k, v in inputs.items()}
    shared = _prep_shared(d)
    x = np.asarray(d["x"], np.float32)

    nc = _get_bass()
    in_maps = []
    for c in range(NCORES):
        m = dict(shared)
        m["xcol"] = _bf(_im2col(x[c]))
        in_maps.append(m)

    import os
    trace = bool(os.environ.get("DCAPS_TRACE"))
    res = run_bass_kernel_spmd(
        nc, in_maps, core_ids=list(range(NCORES)), trace=trace)
    _CACHED["last_results"] = res
    _CACHED["last_in_maps"] = in_maps

    out = np.empty((NCORES, 4, 4, N_CLASSES, 21), np.float32)
    for c in range(NCORES):
        r = np.asarray(res.results[c]["out"])  # (16, 903) o-major: col = o*43+k
        for gy in range(4):
            for gx in range(4):
                cell = (gy >> 1) * 8 + (gy & 1) * 4 + gx
                out[c, gy, gx] = r[cell].reshape(21, N_CLASSES).T
    return out



# revision 29
# speedup vs baseline: 1.2656x; 1.0206x over previous
"""DarkCapsuleNet on 8 Trainium2 NeuronCores.

Data-parallel over batch (B=8, one image per core). The conv+BN+LReLU
backbone runs per core on its image; BN batch statistics are combined
across cores with tiny AllReduces (per-channel [mean, E[x^2]] sums). The
capsule-routing stage is independent per (grid-cell, image), so each core
routes its own 16 cells entirely in SBUF.

Convs are direct convolutions: matmuls accumulated over kernel offsets with
input channels on the contraction dim, bf16 operands, fp32 PSUM. Priors use
a block-diagonal lhsT built on-chip with one masked DVE multiply per tile,
so the 8-wide capsule contraction still runs as full 128-wide matmuls.
"""

import numpy as np
import ml_dtypes


class _PhaseStop(Exception):
    def __init__(self, nc):
        self.nc = nc

N_CLASSES = 43
KO = N_CLASSES * 21  # 903
EPS = 1e-5
NCORES = 8

_BF16 = ml_dtypes.bfloat16


# ---------------------------------------------------------------------------
# Workaround: this walrus build accepts at most ONE sem wait on a TPB_CTRL
# Drain instruction; Tile's epilogue drain carries one wait per HW-DMA queue.
# Split the extra waits onto standalone SP nops (same engine, before the
# all-engine barrier, so semantics are unchanged).
# ---------------------------------------------------------------------------
def _install_tile_drain_fix():
    import concourse.tile as tile_mod
    import concourse.mybir as mybir
    from concourse.vector_clock import ScopedClock

    if getattr(tile_mod.TileContext, "_drain_fix_installed", False):
        return

    def _patched(self, tick_clock, wait_clock):
        drain_inst = self.nc.sync.drain()
        wait_clock.add_sem_waits(
            drain_inst.ins, ScopedClock({None: tick_clock.global_clock})
        )
        raw = drain_inst.ins
        si = getattr(raw, "sync_info", None)
        if si is not None and si.on_wait is not None and len(si.on_wait) > 1:
            waits = list(si.on_wait)
            si.on_wait = waits[-1:]
            for w in waits[:-1]:
                nop = self.nc.sync.nop(nofuse=True, hint="split_drain_wait")
                nsi = getattr(nop.ins, "sync_info", None)
                if nsi is None:
                    nop.ins.sync_info = mybir.SyncInfo(on_update=[], on_wait=[w])
                else:
                    nw = list(nsi.on_wait) if nsi.on_wait else []
                    nw.append(w)
                    nsi.on_wait = nw
        self.nc.all_engine_barrier()
        assert self.sems is not None
        popped = self.nc._tile_sem_poison_stack.pop()
        assert popped is self._sem_poison
        self.nc.clear_and_free_semaphores(list(self.sems.allocated().values()))
        self.nc.all_engine_barrier()

    tile_mod.TileContext._drain_and_barrier = _patched
    tile_mod.TileContext._drain_fix_installed = True


# ---------------------------------------------------------------------------
# Host-side layout prep
# ---------------------------------------------------------------------------
def _bf(x):
    return np.ascontiguousarray(np.asarray(x, np.float32).astype(_BF16))


def _im2col(img):
    # img (3,128,128) f32 -> (27,16384), rows (ci,ky,kx)
    xp = np.zeros((3, 130, 130), np.float32)
    xp[:, 1:129, 1:129] = img
    cols = np.empty((3, 3, 3, 128, 128), np.float32)
    for ky in range(3):
        for kx in range(3):
            cols[:, ky, kx] = xp[:, ky : ky + 128, kx : kx + 128]
    return cols.reshape(27, 16384)


def _prep_shared(d):
    c1h = np.asarray(d["c1w"], np.float32).reshape(128, 27).T.copy()
    c2h = np.asarray(d["c2w"], np.float32).transpose(2, 3, 1, 0).reshape(9, 128, 256)
    c2h = np.concatenate(list(c2h), axis=1)  # (128, 9*256)
    c3t = np.asarray(d["c3w"], np.float32).transpose(1, 2, 3, 0)  # (256,4,4,64)
    c3h = np.concatenate(
        [c3t[m * 128 : (m + 1) * 128].reshape(128, 16 * 64) for m in range(2)], axis=1
    )  # (128, 2048)
    c4h = np.asarray(d["c4w"], np.float32).transpose(1, 2, 3, 0).reshape(64, 16 * 128)
    c5h = np.asarray(d["c5w"], np.float32).transpose(1, 2, 3, 0).reshape(128, 16 * 256)

    rw = np.asarray(d["rw"], np.float32)  # (512,43,8,21)
    # o-major columns: col = o*43 + k (so k is innermost => packed 2-byte
    # innermost dims everywhere in routing => DVE 2x perf mode)
    rt = rw.transpose(0, 2, 3, 1).reshape(512 * 8, KO)  # row = n*8+i, col = o*43+k

    gb = np.zeros((128, 14), np.float32)
    gb[:, 0] = d["g1"]; gb[:, 1] = d["b1"]
    gb[:, 2] = d["g2"][:128]; gb[:, 3] = d["b2"][:128]
    gb[:, 4] = d["g2"][128:]; gb[:, 5] = d["b2"][128:]
    gb[:64, 6] = d["g3"]; gb[:64, 7] = d["b3"]
    gb[:, 8] = d["g4"]; gb[:, 9] = d["b4"]
    gb[:, 10] = d["g5"][:128]; gb[:, 11] = d["b5"][:128]
    gb[:, 12] = d["g5"][128:]; gb[:, 13] = d["b5"][128:]

    mask = np.zeros((128, 128), np.float32)
    for p in range(128):
        mask[p, (p >> 3) * 8 : (p >> 3) * 8 + 8] = 1.0
    selb = np.zeros((128, 8), np.float32)
    for p in range(128):
        selb[p, p & 7] = 1.0
    selr = np.zeros((8, 128), np.float32)  # [b, ns*8 + b]
    for ns in range(16):
        for b in range(8):
            selr[b, ns * 8 + b] = 1.0
    def _f16(x):
        return np.ascontiguousarray(np.asarray(x, np.float16))
    return dict(
        c1wT=_bf(c1h), c2wT=_bf(c2h), c3wT=_bf(c3h), c4wT=_bf(c4h), c5wT=_bf(c5h),
        RT=_f16(rt), gb=gb, MASK=_f16(mask), SELB=_f16(selb),
        SELB43=_f16(selb / 43.0), SELR=_f16(selr),
    )


# ---------------------------------------------------------------------------
# Bass program (identical on every core)
# ---------------------------------------------------------------------------
def _spill_extra_waits(nc):
    """This walrus codegen accepts at most one semaphore wait per TPB
    instruction. Tile can attach several. Move the extras onto fresh NoOp
    instructions inserted just before the owner on the same engine."""
    import concourse.mybir as mybir

    uid = [0]
    for f in nc.m.functions:
        for bb in f.blocks:
            il = bb.instructions
            out = []
            changed = False
            for inst in il:
                si = getattr(inst, "sync_info", None)
                waits = list(si.on_wait) if si is not None and si.on_wait else []
                if len(waits) > 1:
                    for w in waits[:-1]:
                        uid[0] += 1
                        nop = mybir.InstNoOp(name=f"waitspill-{uid[0]}", ins=[], outs=[])
                        nop.engine = inst.engine
                        nop.sync_info = mybir.SyncInfo(on_update=[], on_wait=[w])
                        out.append(nop)
                    si.on_wait = waits[-1:]
                    changed = True
                out.append(inst)
            if changed:
                bb.instructions = out


def _elide_redundant_ldweights(nc):
    """Drop an InstLdweights when the immediately-preceding PE instructions
    were [Ldweights, Matmult] with an identical weights AP and the new
    Ldweights carries no semaphore wait (no wait => the tile framework saw no
    intervening write to the weights buffer, so the PE array already holds
    these weights). Any sem updates move onto the following Matmult."""

    def sig(inst):
        a = inst.ins[0]
        return (a.memref, a.offset, tuple(map(tuple, a.ap)), a.dtype)

    for f in nc.m.functions:
        for bb in f.blocks:
            il = bb.instructions
            last_sig = None
            pending_updates = []
            drop = set()
            for i, inst in enumerate(il):
                if getattr(inst, "engine", None) != inst.engine.__class__.PE:
                    continue
                tn = type(inst).__name__
                if tn == "InstLdweights":
                    si = inst.sync_info
                    waits = list(si.on_wait) if si is not None and si.on_wait else []
                    s = sig(inst)
                    if s == last_sig and not waits:
                        drop.add(i)
                        if si is not None and si.on_update:
                            pending_updates.extend(si.on_update)
                    else:
                        last_sig = s
                elif tn == "InstMatmult":
                    if pending_updates:
                        si = inst.sync_info
                        if si is None:
                            import concourse.mybir as mybir
                            inst.sync_info = mybir.SyncInfo(
                                on_update=list(pending_updates), on_wait=[])
                        else:
                            upd = list(si.on_update) if si.on_update else []
                            upd.extend(pending_updates)
                            si.on_update = upd
                        pending_updates = []
                else:
                    last_sig = None
            assert not pending_updates
            if drop:
                bb.instructions = [x for i, x in enumerate(il) if i not in drop]


def _build_bass(phase_limit=99):
    import concourse.bass as bass
    import concourse.mybir as mybir
    from concourse import tile

    _install_tile_drain_fix()

    F32 = mybir.dt.float32
    BF16 = mybir.dt.bfloat16
    F16 = mybir.dt.float16
    ADD = mybir.AluOpType.add
    MULT = mybir.AluOpType.mult
    SUB = mybir.AluOpType.subtract
    ACTF = mybir.ActivationFunctionType
    AXX = mybir.AxisListType.X

    nc = bass.Bass(num_devices=NCORES)
    dp = nc.declare_dram_parameter
    i_xcol = dp("xcol", [27, 16384], BF16, isOutput=False)
    i_c1 = dp("c1wT", [27, 128], BF16, isOutput=False)
    i_c2 = dp("c2wT", [128, 2304], BF16, isOutput=False)
    i_c3 = dp("c3wT", [128, 2048], BF16, isOutput=False)
    i_c4 = dp("c4wT", [64, 2048], BF16, isOutput=False)
    i_c5 = dp("c5wT", [128, 4096], BF16, isOutput=False)
    i_rt = dp("RT", [4096, KO], F16, isOutput=False)
    i_gb = dp("gb", [128, 14], F32, isOutput=False)
    i_mask = dp("MASK", [128, 128], F16, isOutput=False)
    i_selb = dp("SELB", [128, 8], F16, isOutput=False)
    i_selb43 = dp("SELB43", [128, 8], F16, isOutput=False)
    i_selr = dp("SELR", [8, 128], F16, isOutput=False)
    o_out = dp("out", [16, KO], F32, isOutput=True)


    with tile.TileContext(nc) as tc:
        with tc.tile_pool(name="const", bufs=1) as const, \
             tc.tile_pool(name="dram", bufs=1, space="DRAM") as dram:
            t_gb = const.tile([128, 14], F32)
            t_mask = const.tile([128, 128], F16)
            t_selb = const.tile([128, 8], F16)
            t_selb43 = const.tile([128, 8], F16)
            t_selr = const.tile([8, 128], F16)
            h5 = [const.tile([128, 256], BF16, tag=f"h5_{m}", name=f"h5_{m}") for m in range(2)]
            t_st6 = const.tile([128, 32 * 6], F32)
            t_mv = const.tile([128, 8], F32)
            t_ab = const.tile([128, 4], F32)
            t_sc = const.tile([128, 2], F32)
            for t, i in [(t_gb, i_gb), (t_mask, i_mask), (t_selb, i_selb),
                         (t_selb43, i_selb43), (t_selr, i_selr)]:
                nc.sync.dma_start(t[:], i[:])

            ar_in = [dram.tile([128, 2], F32, tag=f"ari{i}", name=f"ari{i}") for i in range(7)]
            ar_out = [dram.tile([8, 256], F32, tag=f"aro{i}", name=f"aro{i}") for i in range(7)]
            t_ag = const.tile([128, 32], F32)

            def bn_ag_start(slot, mt, npart):
                """AllGather the [mean, E[x^2]] stats in t_mv[:, 2mt:2mt+2]."""
                m = t_mv[:npart, 2 * mt : 2 * mt + 1]
                v = t_mv[:npart, 2 * mt + 1 : 2 * mt + 2]
                s1 = t_sc[:npart, 0:1]
                nc.scalar.activation(s1, m, ACTF.Square)
                nc.vector.tensor_tensor(v, v, s1, ADD)  # v := E[x^2] local
                nc.sync.dma_start(ar_in[slot][:], t_mv[:, 2 * mt : 2 * mt + 2])
                nc.gpsimd.collective_compute(
                    "AllGather", mybir.AluOpType.bypass,
                    ins=[ar_in[slot][:]], outs=[ar_out[slot][:]],
                    replica_groups=[list(range(NCORES))],
                )

            def bn_ag_finish(slot, gcol, mt, npart):
                """Reduce gathered stats, write [a,b] into t_ab[:, 2mt:2mt+2]."""
                agv = t_ag[:, 16 * (slot & 1) : 16 * (slot & 1) + 16]
                nc.sync.dma_start(
                    agv.rearrange("p (g c) -> p g c", c=2),
                    ar_out[slot][:].rearrange("g (p c) -> p g c", c=2),
                )
                mq = t_mv[:, 4 + 2 * (slot & 1) : 6 + 2 * (slot & 1)]
                nc.vector.tensor_reduce(
                    mq, agv.rearrange("p (g c) -> p c g", c=2), AXX, ADD)
                m = mq[:npart, 0:1]
                q = mq[:npart, 1:2]
                a = t_ab[:npart, 2 * mt : 2 * mt + 1]
                b = t_ab[:npart, 2 * mt + 1 : 2 * mt + 2]
                s1 = t_sc[:npart, 1:2]
                nc.vector.tensor_scalar_mul(m, m, 1.0 / NCORES)
                nc.vector.tensor_scalar_mul(q, q, 1.0 / NCORES)
                nc.scalar.activation(s1, m, ACTF.Square)
                nc.vector.tensor_tensor(q, q, s1, SUB)       # gvar
                nc.vector.tensor_scalar_add(q, q, EPS)
                nc.vector.reciprocal(s1, q)
                nc.scalar.activation(s1, s1, ACTF.Sqrt)      # rsqrt(var+eps)
                nc.vector.tensor_tensor(a, t_gb[:npart, gcol : gcol + 1], s1, MULT)
                nc.vector.tensor_tensor(s1, a, m, MULT)
                nc.vector.tensor_tensor(b, t_gb[:npart, gcol + 1 : gcol + 2], s1, SUB)

            def bn_allreduce(layer, nch_tiles, npart):
                """Combined start+finish for single-sync layers."""
                slot, gcol = {0: (0, 0), 2: (3, 6), 3: (4, 8)}[layer]
                for mt in range(nch_tiles):
                    bn_ag_start(slot + mt, mt, npart)
                for mt in range(nch_tiles):
                    bn_ag_finish(slot + mt, gcol + 2 * mt, mt, npart)

            def lrelu_apply(view, scale, bias):
                nc.scalar.activation(view, view, ACTF.Prelu,
                                     bias=bias, scale=scale, alpha=0.1)

            # ================= conv backbone =================
            with tc.tile_pool(name="wpool", bufs=1) as wp, \
                 tc.tile_pool(name="xpool", bufs=1) as xp, \
                 tc.tile_pool(name="acts", bufs=1) as acts, \
                 tc.tile_pool(name="cpsum", bufs=4, space="PSUM") as cpsum:
                t_c2 = wp.tile([128, 2304], BF16)
                t_c3 = wp.tile([128, 2048], BF16)
                t_c4 = wp.tile([64, 2048], BF16)
                t_c5 = wp.tile([128, 4096], BF16)
                t_c1 = xp.tile([27, 128], BF16)
                t_xcol = xp.tile([27, 16384], BF16)
                nc.sync.dma_start(t_c1[:], i_c1[:])
                for ch in range(4):
                    nc.sync.dma_start(t_xcol[:, ch * 4096 : (ch + 1) * 4096],
                                      i_xcol[:, ch * 4096 : (ch + 1) * 4096])

                h1 = acts.tile([128, 130 * 130], BF16)
                h2 = [acts.tile([128, 130 * 130], BF16, tag=f"h2_{m}", name=f"h2_{m}") for m in range(2)]
                h3 = acts.tile([64, 66 * 66], BF16)
                h4 = acts.tile([128, 34 * 34], BF16)

                def zero_border(tile_ap, H):
                    v = tile_ap.rearrange("p (a b) -> p a b", b=H)
                    nc.gpsimd.memset(v[:, 0:1, :], 0.0)
                    nc.gpsimd.memset(v[:, H - 1 : H, :], 0.0)
                    nc.gpsimd.memset(v[:, 1 : H - 1, 0:1], 0.0)
                    nc.gpsimd.memset(v[:, 1 : H - 1, H - 1 : H], 0.0)

                zero_border(h1[:], 130)
                zero_border(h2[0][:], 130)
                zero_border(h2[1][:], 130)
                zero_border(h3[:], 66)
                zero_border(h4[:], 34)

                # ---- conv1 ----
                for nt in range(32):
                    ps = cpsum.tile([128, 512], F32, tag="cps")
                    nc.tensor.matmul(ps[:], t_c1[:],
                                     t_xcol[:, nt * 512 : (nt + 1) * 512],
                                     start=True, stop=True)
                    intr = h1[:].rearrange("p (a b) -> p a b", b=130)[
                        :, 1 + nt * 4 : 5 + nt * 4, 1:129]
                    nc.scalar.activation(
                        intr, ps[:].rearrange("p (a b) -> p a b", b=128), ACTF.Copy)
                    nc.vector.bn_stats(t_st6[:, nt * 6 : nt * 6 + 6], ps[:])
                for t, i in [(t_c2, i_c2), (t_c3, i_c3), (t_c4, i_c4),
                             (t_c5, i_c5)]:
                    nc.sync.dma_start(t[:], i[:])
                nc.vector.bn_aggr(t_mv[:, 0:2],
                                  t_st6[:].rearrange("p (g s) -> p g s", s=6))
                bn_allreduce(0, 1, 128)
                h1v = h1[:].rearrange("p (a b) -> p a b", b=130)
                for c4_ in range(4):
                    lrelu_apply(h1v[:, 1 + 32 * c4_ : 33 + 32 * c4_, 1:129],
                                t_ab[:, 0:1], t_ab[:, 1:2])

                # ---- conv2 ----
                if phase_limit < 2:
                    raise _PhaseStop(nc)
                for m in range(2):
                    for nt in range(32):
                        ps = cpsum.tile([128, 512], F32, tag="cps")
                        for off in range(9):
                            ky, kx = off // 3, off % 3
                            rhs = h1v[:, ky + nt * 4 : ky + nt * 4 + 4, kx : kx + 128]
                            nc.tensor.matmul(
                                ps[:],
                                t_c2[:, off * 256 + m * 128 : off * 256 + m * 128 + 128],
                                rhs, start=(off == 0), stop=(off == 8))
                        intr = h2[m][:].rearrange("p (a b) -> p a b", b=130)[
                            :, 1 + nt * 4 : 5 + nt * 4, 1:129]
                        nc.scalar.activation(
                            intr, ps[:].rearrange("p (a b) -> p a b", b=128), ACTF.Copy)
                        nc.vector.bn_stats(t_st6[:, nt * 6 : nt * 6 + 6], ps[:])
                    nc.vector.bn_aggr(t_mv[:, 2 * m : 2 * m + 2],
                                      t_st6[:].rearrange("p (g s) -> p g s", s=6))
                    # AG for this half flies while the other half computes
                    bn_ag_start(1 + m, m, 128)
                h2v = [h2[m][:].rearrange("p (a b) -> p a b", b=130) for m in range(2)]
                bn_ag_finish(1, 2, 0, 128)
                for c4_ in range(4):
                    lrelu_apply(h2v[0][:, 1 + 32 * c4_ : 33 + 32 * c4_, 1:129],
                                t_ab[:, 0:1], t_ab[:, 1:2])

                # ---- conv3 ----
                # nt 0..3: open m0-only partial sums to overlap conv2's second
                # stats AllGather, then finish with m1 after h2[1] is BN'd.
                if phase_limit < 3:
                    raise _PhaseStop(nc)
                c3ps = []
                for nt in range(4):
                    ps = cpsum.tile([128, 512], F32, tag="cps")
                    c3ps.append(ps)
                    for off in range(16):
                        ky, kx = off // 4, off % 4
                        rhs = h2v[0][:, ky + nt * 16 : ky + nt * 16 + 15 : 2,
                                     kx : kx + 127 : 2]
                        nc.tensor.matmul(
                            ps[:64, :], t_c3[:, off * 64 : off * 64 + 64],
                            rhs, start=(off == 0), stop=False,
                            skip_group_check=True)
                bn_ag_finish(2, 4, 1, 128)
                for c4_ in range(4):
                    lrelu_apply(h2v[1][:, 1 + 32 * c4_ : 33 + 32 * c4_, 1:129],
                                t_ab[:, 2:3], t_ab[:, 3:4])

                def c3_finish(nt, ps):
                    intr = h3[:].rearrange("p (a b) -> p a b", b=66)[
                        :, 1 + nt * 8 : 9 + nt * 8, 1:65]
                    nc.scalar.activation(
                        intr, ps[:64, :].rearrange("p (a b) -> p a b", b=64), ACTF.Copy)
                    nc.vector.bn_stats(t_st6[:64, nt * 6 : nt * 6 + 6], ps[:64, :])

                for nt in range(4):
                    ps = c3ps[nt]
                    for off in range(16):
                        ky, kx = off // 4, off % 4
                        rhs = h2v[1][:, ky + nt * 16 : ky + nt * 16 + 15 : 2,
                                     kx : kx + 127 : 2]
                        nc.tensor.matmul(
                            ps[:64, :], t_c3[:, (16 + off) * 64 : (16 + off) * 64 + 64],
                            rhs, start=False, stop=(off == 15),
                            skip_group_check=True)
                    c3_finish(nt, ps)
                for nt in range(4, 8):
                    ps = cpsum.tile([128, 512], F32, tag="cps")
                    first = True
                    for m in range(2):
                        for off in range(16):
                            ky, kx = off // 4, off % 4
                            rhs = h2v[m][:, ky + nt * 16 : ky + nt * 16 + 15 : 2,
                                         kx : kx + 127 : 2]
                            nc.tensor.matmul(
                                ps[:64, :],
                                t_c3[:, (m * 16 + off) * 64 : (m * 16 + off) * 64 + 64],
                                rhs, start=first, stop=(m == 1 and off == 15))
                            first = False
                    c3_finish(nt, ps)
                nc.vector.bn_aggr(
                    t_mv[:64, 0:2],
                    t_st6[:64, : 8 * 6].rearrange("p (g s) -> p g s", s=6))
                bn_allreduce(2, 1, 64)
                h3v = h3[:].rearrange("p (a b) -> p a b", b=66)
                lrelu_apply(h3v[:, 1:65, 1:65], t_ab[:64, 0:1], t_ab[:64, 1:2])

                # ---- conv4 ----
                if phase_limit < 4:
                    raise _PhaseStop(nc)
                for nt in range(2):
                    ps = cpsum.tile([128, 512], F32, tag="cps")
                    for off in range(16):
                        ky, kx = off // 4, off % 4
                        rhs = h3v[:, ky + nt * 32 : ky + nt * 32 + 31 : 2, kx : kx + 63 : 2]
                        nc.tensor.matmul(ps[:], t_c4[:, off * 128 : off * 128 + 128],
                                         rhs, start=(off == 0), stop=(off == 15))
                    intr = h4[:].rearrange("p (a b) -> p a b", b=34)[
                        :, 1 + nt * 16 : 17 + nt * 16, 1:33]
                    nc.scalar.activation(
                        intr, ps[:].rearrange("p (a b) -> p a b", b=32), ACTF.Copy)
                    nc.vector.bn_stats(t_st6[:, nt * 6 : nt * 6 + 6], ps[:])
                nc.vector.bn_aggr(
                    t_mv[:, 0:2], t_st6[:, :12].rearrange("p (g s) -> p g s", s=6))
                bn_allreduce(3, 1, 128)
                h4v = h4[:].rearrange("p (a b) -> p a b", b=34)
                lrelu_apply(h4v[:, 1:33, 1:33], t_ab[:, 0:1], t_ab[:, 1:2])

                # ---- conv5 ----
                if phase_limit < 5:
                    raise _PhaseStop(nc)
                for m in range(2):
                    ps = cpsum.tile([128, 512], F32, tag="cps")
                    first = True
                    for off in range(16):
                        ky, kx = off // 4, off % 4
                        rhs = h4v[:, ky : ky + 31 : 2, kx : kx + 31 : 2]
                        nc.tensor.matmul(
                            ps[:, 0:256],
                            t_c5[:, off * 256 + m * 128 : off * 256 + m * 128 + 128],
                            rhs, start=first, stop=(off == 15))
                        first = False
                    nc.scalar.activation(h5[m][:], ps[:, 0:256], ACTF.Copy)
                    nc.vector.bn_stats(t_st6[:, m * 6 : m * 6 + 6], ps[:, 0:256])
                for m in range(2):
                    nc.vector.bn_aggr(
                        t_mv[:, 2 * m : 2 * m + 2],
                        t_st6[:, m * 6 : m * 6 + 6].rearrange("p (g s) -> p g s", s=6))
                    bn_ag_start(5 + m, m, 128)
                bn_ag_finish(5, 10, 0, 128)
                lrelu_apply(h5[0][:], t_ab[:, 0:1], t_ab[:, 1:2])
                # h5[1]'s finish + lrelu are emitted inside the priors loop
                # (after several h5[0]-only tiles) to overlap its AllGather

            if phase_limit < 6:
                raise _PhaseStop(nc)
            # ================= priors (o-major: free = t*903 + o*43 + k) ====
            with tc.tile_pool(name="pri", bufs=1) as pri, \
                 tc.tile_pool(name="route", bufs=1) as rp, \
                 tc.tile_pool(name="scr", bufs=4) as scr:
                P = [[pri.tile([128, 8 * KO], F16, tag=f"P{g}_{j}", name=f"P{g}_{j}")
                      for j in range(4)] for g in range(2)]
                NG = 4   # tile-groups per cell-group (8 tiles each)
                GT = 8
                L = [[rp.tile([128, GT * 43], F16, tag=f"L{g}_{j}", name=f"L{g}_{j}")
                      for j in range(NG)] for g in range(2)]
                s_g = [rp.tile([8, KO], F32, tag=f"s_g{g}", name=f"s_g{g}") for g in range(2)]
                sn = [rp.tile([8, 43], F32, tag=f"sn{g}", name=f"sn{g}") for g in range(2)]
                den = [rp.tile([8, 43], F32, tag=f"den{g}", name=f"den{g}") for g in range(2)]
                phi = [rp.tile([8, 43], F32, tag=f"phi{g}", name=f"phi{g}") for g in range(2)]
                out_f = [rp.tile([8, KO], F32, tag=f"of{g}", name=f"of{g}") for g in range(2)]
                out_bf = [rp.tile([8, KO], F16, tag=f"ob{g}", name=f"ob{g}") for g in range(2)]
                out_rep = [rp.tile([128, KO], BF16, tag=f"orep{g}", name=f"orep{g}") for g in range(2)]
                for g in range(2):
                    for j in range(NG):
                        nc.vector.memset(L[g][j][:], 0.0)

                sp0 = [None, None]
                with tc.tile_pool(name="ppsum", bufs=1, space="PSUM") as ppsum:
                    cpy = 0
                    t_order = [t for t in range(32) if not (t & 1)] + \
                              [t for t in range(32) if t & 1]
                    for ti, t in enumerate(t_order):
                        if ti == 8:
                            # conv5's second-half AllGather has been in flight
                            # behind the first 8 h5[0]-tiles; land it now
                            bn_ag_finish(6, 12, 1, 128)
                            lrelu_apply(h5[1][:], t_ab[:, 2:3], t_ab[:, 3:4])
                        h = t >> 3
                        w = (t >> 1) & 3
                        mblk = t & 1
                        j, tj = t // GT, t % GT
                        rt_t = scr.tile([128, KO], F16, tag="rt", bufs=3)
                        nc.sync.dma_start(rt_t[:], i_rt[t * 128 : (t + 1) * 128, :])
                        hb = h5[mblk][:].rearrange(
                            "p (hh gy gx ww) -> p hh gy gx ww",
                            hh=4, gy=4, gx=4)
                        for g in range(2):
                            g8 = scr.tile([128, 8], F16, tag="g8")
                            src = hb[:, h : h + 1, 2 * g : 2 * g + 2, :, w : w + 1]
                            # (p,1,2,4,1) -> (p,2,4)
                            src = src.rearrange("p a b d e -> p (a b) (d e)")
                            nc.gpsimd.tensor_copy(
                                g8[:].rearrange("p (b d) -> p b d", b=2), src)
                            lt = scr.tile([128, 128], F16, tag="lt", bufs=2)
                            nc.vector.tensor_tensor(
                                lt[:].rearrange("p (n b) -> p n b", b=8),
                                g8[:].rearrange("p (o e) -> p o e", o=1)
                                    .broadcast_to([128, 16, 8]),
                                t_mask[:].rearrange("p (n b) -> p n b", b=8),
                                MULT)
                            pp = ppsum.tile([128, KO], F32, tag="pps", bufs=2)
                            nc.tensor.matmul(pp[:, 0:512], lt[:], rt_t[:, 0:512],
                                             start=True, stop=True,
                                             skip_group_check=True)
                            nc.tensor.matmul(pp[:, 512:KO], lt[:], rt_t[:, 512:KO],
                                             start=True, stop=True,
                                             skip_group_check=True)
                            dst = P[g][j][:, tj * KO : tj * KO + KO]
                            # rotate PSUM->SBUF copies across Act/DVE
                            # (GPSIMD/Pool cannot access PSUM; Act is cheaper
                            # than DVE here since f32 psum reads get no 2x)
                            if cpy % 3 == 1:
                                nc.vector.tensor_copy(dst, pp[:])
                            else:
                                nc.scalar.activation(dst, pp[:], ACTF.Copy)
                            cpy += 1
                            # it0 s-sum: probs are uniform 1/43
                            if t == 0:
                                sp0[g] = ppsum.tile([8, KO], F32, tag=f"sp0_{g}",
                                                    bufs=1, name=f"sp0_{g}")
                            nc.tensor.matmul(sp0[g][:, 0:512], t_selb43[:],
                                             dst[:, 0:512],
                                             start=(t == 0), stop=(t == 31),
                                             skip_group_check=True)
                            nc.tensor.matmul(sp0[g][:, 512:KO], t_selb43[:],
                                             dst[:, 512:KO],
                                             start=(t == 0), stop=(t == 31),
                                             skip_group_check=True)
                    for g in range(2):
                        nc.scalar.activation(s_g[g][:], sp0[g][:], ACTF.Copy)

                # ================= routing =================
                if phase_limit < 7:
                    raise _PhaseStop(nc)

                def squash(g, it, rpsum):
                    """out = s * sqrt(sn)/(1+sn); free dim o-major (o,k)."""
                    nc.scalar.activation(out_f[g][:], s_g[g][:], ACTF.Square)
                    nc.vector.tensor_reduce(
                        sn[g][:], out_f[g][:].rearrange("p (o k) -> p k o", k=43),
                        AXX, ADD)
                    nc.vector.tensor_scalar_add(den[g][:], sn[g][:], 1.0)
                    nc.vector.reciprocal(den[g][:], den[g][:])
                    nc.scalar.activation(phi[g][:], sn[g][:], ACTF.Sqrt)
                    nc.vector.tensor_tensor(phi[g][:], phi[g][:], den[g][:], MULT)
                    tgt = out_f[g] if it == 2 else out_bf[g]
                    nc.vector.tensor_tensor(
                        tgt[:].rearrange("p (o k) -> p o k", k=43),
                        s_g[g][:].rearrange("p (o k) -> p o k", k=43),
                        phi[g][:].rearrange("p (o k) -> p o k", o=1)
                              .broadcast_to([8, 21, 43]),
                        MULT)  # phi is [8,43]: o=1 split then bcast over o
                    if it == 2:
                        nc.sync.dma_start(o_out[g * 8 : g * 8 + 8, :], tgt[:])
                    else:
                        rpp = rpsum.tile([128, KO], F32, tag="rep", bufs=2)
                        nc.tensor.matmul(
                            rpp[:, 0:512], t_selr[:],
                            out_bf[g][:, 0:512], start=True, stop=True)
                        nc.tensor.matmul(
                            rpp[:, 512:KO], t_selr[:],
                            out_bf[g][:, 512:KO], start=True, stop=True)
                        nc.scalar.activation(out_rep[g][:], rpp[:], ACTF.Copy)

                # Pool (gpsimd) takes a fixed slice of the t-tiles of every big
                # multiply; DVE (2x perf mode, ~3.8x faster per elem) takes the
                # rest plus the tree reduce. Fine slices => tight overlap.
                AP_POOL_T = 2   # pool gets t-tiles [6:8] of each ap
                TM_POOL_T = 3   # pool gets t-tiles [5:8] of each tm

                with tc.tile_pool(name="rpsum", bufs=1, space="PSUM") as rpsum:
                    def emit_ap(g, j):
                        """ap = P * out (f16), bcast out over t; pool takes the
                        last AP_POOL_T t-tiles, DVE the rest."""
                        Pj = P[g][j][:].rearrange(
                            "p (t o k) -> p t o k", o=21, k=43)
                        ap = scr.tile([128, 8 * KO], F16, tag="big", bufs=4,
                                      name=f"ap{g}{j}")
                        ap4 = ap[:].rearrange("p (t o k) -> p t o k", o=21, k=43)
                        orr = out_rep[g][:].rearrange(
                            "p (a o k) -> p a o k", a=1, k=43)
                        ts = 8 - AP_POOL_T
                        with nc.allow_low_precision("logit delta fp16"):
                            nc.gpsimd.tensor_tensor(
                                ap4[:, ts:8], Pj[:, ts:8],
                                orr.broadcast_to([128, AP_POOL_T, 21, 43]), MULT)
                            nc.vector.tensor_tensor(
                                ap4[:, 0:ts], Pj[:, 0:ts],
                                orr.broadcast_to([128, ts, 21, 43]), MULT)
                        return ap

                    # ap(g,j) depends only on out_rep[g], which the previous
                    # iteration's squash(g) produced — so each phase's first
                    # two aps are emitted BEFORE the preceding squash, keeping
                    # the in-order DVE queue busy across phase transitions.
                    squash(0, 0, rpsum)
                    aps_next = [emit_ap(0, 0), emit_ap(0, 1)]
                    squash(1, 0, rpsum)
                    phases = [(1, 0), (1, 1), (2, 0), (2, 1)]
                    for pi, (it, g) in enumerate(phases):
                        if True:
                            sp = rpsum.tile([8, KO], F32, tag="sps", bufs=2)
                            aps = aps_next
                            for j in range(NG):
                                Pj = P[g][j][:].rearrange(
                                    "p (t o k) -> p t o k", o=21, k=43)
                                ap4 = aps[j][:].rearrange(
                                    "p (t o k) -> p t o k", o=21, k=43)
                                with nc.allow_low_precision("logit delta fp16"):
                                    # in-place tree reduce over o (21 = 10+10+1)
                                    for lo, w2 in ((0, 10), (0, 5),
                                                   (0, 2), (0, 1)):
                                        nc.vector.tensor_tensor(
                                            ap4[:, :, lo : lo + w2, :],
                                            ap4[:, :, lo : lo + w2, :],
                                            ap4[:, :, lo + w2 : lo + 2 * w2, :],
                                            ADD)
                                    nc.vector.tensor_tensor(
                                        ap4[:, :, 0:1, :], ap4[:, :, 0:1, :],
                                        ap4[:, :, 4:5, :], ADD)
                                    nc.vector.tensor_tensor(
                                        ap4[:, :, 0:1, :], ap4[:, :, 0:1, :],
                                        ap4[:, :, 20:21, :], ADD)
                                    # L += delta
                                    L4 = L[g][j][:].rearrange(
                                        "p (t a k) -> p t a k", a=1, k=43)
                                    nc.vector.tensor_tensor(
                                        L4, L4, ap4[:, :, 0:1, :], ADD)
                                # softmax over k
                                e8 = scr.tile([128, GT * 43], F16, tag="e8", bufs=2)
                                nc.scalar.activation(e8[:], L[g][j][:], ACTF.Exp)
                                r8 = scr.tile([128, GT], F32, tag="r8", bufs=2)
                                nc.vector.tensor_reduce(
                                    r8[:], e8[:].rearrange("p (t k) -> p t k", k=43),
                                    AXX, ADD)
                                nc.vector.reciprocal(r8[:], r8[:])
                                nc.vector.tensor_tensor(
                                    e8[:].rearrange("p (t k) -> p t k", k=43),
                                    e8[:].rearrange("p (t k) -> p t k", k=43),
                                    r8[:].rearrange("p (t k) -> p t k", k=1)
                                        .broadcast_to([128, GT, 43]),
                                    MULT)
                                # prefetch ap for j+2 before tm(j) so the
                                # pool never blocks the next tree
                                if j + 2 < NG:
                                    aps.append(emit_ap(g, j + 2))
                                # tm = P * probs (f16), bcast probs over o
                                tm = scr.tile([128, 8 * KO], F16, tag="big",
                                              bufs=4, name=f"tm{g}{j}")
                                tm4 = tm[:].rearrange(
                                    "p (t o k) -> p t o k", o=21, k=43)
                                prr = e8[:].rearrange(
                                    "p (t a k) -> p t a k", a=1, k=43)
                                # last j-group: all on DVE so the pool is free
                                # to run the next phase's prefetched aps
                                tpool = 0 if j == NG - 1 else TM_POOL_T
                                ts = 8 - tpool
                                if tpool:
                                    nc.gpsimd.tensor_tensor(
                                        tm4[:, ts:8], Pj[:, ts:8],
                                        prr[:, ts:8].broadcast_to(
                                            [128, tpool, 21, 43]),
                                        MULT)
                                nc.vector.tensor_tensor(
                                    tm4[:, 0:ts], Pj[:, 0:ts],
                                    prr[:, 0:ts].broadcast_to([128, ts, 21, 43]),
                                    MULT)
                                for tj in range(GT):
                                    rhs_t = tm[:, tj * KO : tj * KO + KO]
                                    nc.tensor.matmul(
                                        sp[:, 0:512], t_selb[:], rhs_t[:, 0:512],
                                        start=(j == 0 and tj == 0),
                                        stop=(j == NG - 1 and tj == GT - 1))
                                    nc.tensor.matmul(
                                        sp[:, 512:KO], t_selb[:], rhs_t[:, 512:KO],
                                        start=(j == 0 and tj == 0),
                                        stop=(j == NG - 1 and tj == GT - 1))
                            nc.scalar.activation(s_g[g][:], sp[:], ACTF.Copy)
                            if pi + 1 < len(phases):
                                gn = phases[pi + 1][1]
                                aps_next = [emit_ap(gn, 0), emit_ap(gn, 1)]
                            squash(g, it, rpsum)
    _elide_redundant_ldweights(nc)
    _spill_extra_waits(nc)
    return nc


_CACHED = {}


def _get_bass():
    if "nc" not in _CACHED:
        _CACHED["nc"] = _build_bass()
    return _CACHED["nc"]


def kernel(**inputs):
    from concourse.bass_utils import run_bass_kernel_spmd

    d = {k: np.asarray(v) for k, v in inputs.items()}
    shared = _prep_shared(d)
    x = np.asarray(d["x"], np.float32)

    nc = _get_bass()
    in_maps = []
    for c in range(NCORES):
        m = dict(shared)
        m["xcol"] = _bf(_im2col(x[c]))
        in_maps.append(m)

    import os
    trace = bool(os.environ.get("DCAPS_TRACE"))
    res = run_bass_kernel_spmd(
        nc, in_maps, core_ids=list(range(NCORES)), trace=trace)
    _CACHED["last_results"] = res
    _CACHED["last_in_maps"] = in_maps

    out = np.empty((NCORES, 4, 4, N_CLASSES, 21), np.float32)
    for c in range(NCORES):
        r = np.asarray(res.results[c]["out"])  # (16, 903) o-major: col = o*43+k
        for gy in range(4):
            for gx in range(4):
                cell = (gy >> 1) * 8 + (gy & 1) * 4 + gx
                out[c, gy, gx] = r[cell].reshape(21, N_CLASSES).T
    return out

